# revision 1
# baseline (speedup 1.0000x reference)
"""CPI_DGLLife kernel for 8 Trainium2 NeuronCores (SPMD).

GCN over a 65536-node graph + protein conv1d branch + CPI head.
Sharding: data-parallel over the 512-graph batch (64 graphs / core).
Each core: full h0 table build (replicated), dma_gather edge aggregation
for its dst nodes, fp32r conv stack for its 64 proteins.
"""
import sys
sys.path.insert(0, "/opt/trn_rl_repo")
import contextlib
import numpy as np

import concourse.bass as bass
import concourse.bacc as bacc
import concourse.tile as tile
from concourse import mybir
from concourse.bass_utils import run_bass_kernel_spmd
from concourse.masks import make_identity

dt = mybir.dt
AF = mybir.ActivationFunctionType
ALU = mybir.AluOpType
AX = mybir.AxisListType

P = 128
N, E, B, L = 65536, 262144, 512, 1000
IN_DIM, HID, VOCAB = 74, 128, 25
CHANNELS = [HID, 96, 128, IN_DIM, HID]
NCORES = 8
GPC = B // NCORES              # graphs per core = 64
PPC = GPC                      # proteins per core = 64
# h0 tables: 512-aligned split, local idx = node - base + 1, row 0 = zeros
TBL_BASES = [0, 32256, 64512]
TBL_NNODES = [32256, 32256, 1024]
TBL_ROWS = [n + 1 for n in TBL_NNODES]
TOK_BUDGET = 4096              # max tokens per dma_gather instruction
LCONV = 1002                   # 1000 + 2 guard cols


# ------------------------------------------------------------------ host prep
def _host_prep(inputs):
    graph_ids = np.asarray(inputs["graph_ids"])
    src = np.concatenate([np.asarray(inputs["edge_src"]).astype(np.int64),
                          np.arange(N, dtype=np.int64)])
    dst = np.concatenate([np.asarray(inputs["edge_dst"]).astype(np.int64),
                          np.arange(N, dtype=np.int64)])
    deg_out = np.bincount(src, minlength=N).astype(np.float32)
    deg_in = np.bincount(dst, minlength=N).astype(np.float32)

    core_node_lo = np.searchsorted(graph_ids, np.arange(0, B + 1, GPC))
    ncore_nodes = core_node_lo[1:] - core_node_lo[:-1]
    NT = int(np.ceil(ncore_nodes.max() / P))  # tiles per core (uniform)
    NPAD = NT * P

    # per-core degree-sorted node permutation (padded with -1)
    perm = np.full((NCORES, NPAD), -1, np.int64)
    for c in range(NCORES):
        lo, hi = int(core_node_lo[c]), int(core_node_lo[c + 1])
        order = np.argsort(-deg_in[lo:hi], kind="stable") + lo
        perm[c, :hi - lo] = order

    # deg_in per perm position (pad 1.0), laid out [P, NT] (p, t)
    deg_in_perm = np.ones((NCORES, NPAD), np.float32)
    m = perm >= 0
    deg_in_perm[m] = deg_in[perm[m]]
    deg_in_perm = deg_in_perm.reshape(NCORES, NT, P).transpose(0, 2, 1).copy()

    # S tiles: [NT, P, GPC] graph membership of permuted nodes
    S = np.zeros((NCORES, NT, P, GPC), np.float32)
    for c in range(NCORES):
        pm = perm[c]
        valid = pm >= 0
        g = graph_ids[pm[valid]] - c * GPC
        tt = np.arange(NPAD)[valid] // P
        pp = np.arange(NPAD)[valid] % P
        S[c, tt, pp, g] = 1.0

    # node -> (core, tile-position) in permuted order
    pos_of = np.full(N, -1, np.int64)
    core_of = np.full(N, -1, np.int64)
    for c in range(NCORES):
        pm = perm[c]
        v = pm >= 0
        pos_of[pm[v]] = np.arange(NPAD)[v]
        core_of[pm[v]] = c

    # table id + local row of each node (as gather source)
    tbl_of = np.digitize(np.arange(N), TBL_BASES[1:])
    loc_of = (np.arange(N) - np.asarray(TBL_BASES)[tbl_of] + 1).astype(np.int64)

    # edge placement: core/tile/lane from dst, table/local from src
    ec = core_of[dst]
    et = pos_of[dst] // P
    ep = pos_of[dst] % P
    etbl = tbl_of[src]
    eloc = loc_of[src]

    # slot index within (core, tile, lane, table) group
    key = (((ec * NT + et) * P + ep) * 3 + etbl)
    order = np.argsort(key, kind="stable")
    ks = key[order]
    starts = np.r_[0, np.flatnonzero(np.diff(ks)) + 1]
    grp_len = np.diff(np.r_[starts, E + N])
    slot_sorted = np.arange(E + N) - np.repeat(starts, grp_len)
    slot = np.empty(E + N, np.int64)
    slot[order] = slot_sorted
    # counts per (c, t, p, T) -> kmax per (t, T) across cores/lanes
    cnt = np.zeros(NCORES * NT * P * 3, np.int64)
    uk, uc = np.unique(ks, return_counts=True)
    cnt[uk] = uc
    cnt = cnt.reshape(NCORES, NT, P, 3)
    kmax = cnt.max(axis=2).max(axis=0)  # [NT, 3]

    # gather token schedule per table: tiles packed into instructions
    sched = []  # per table: list of instruction = list of (tile, k)
    for T in range(3):
        instrs, cur, tok = [], [], 0
        for t in range(NT):
            k = int(kmax[t, T])
            if k == 0:
                continue
            if tok + k * P > TOK_BUDGET and cur:
                instrs.append(cur)
                cur, tok = [], 0
            cur.append((t, k))
            tok += k * P
        if cur:
            instrs.append(cur)
        sched.append(instrs)

    # token offset of each tile inside its table stream
    tile_off = np.full((3, NT), 0, np.int64)
    tok_total = [0, 0, 0]
    for T in range(3):
        off = 0
        for ins in sched[T]:
            for (t, k) in ins:
                tile_off[T, t] = off
                off += k * P
        tok_total[T] = max(off, 128)

    idx_flat = [np.zeros((NCORES, tok_total[T]), np.int16) for T in range(3)]
    tok_pos = tile_off[etbl, et] + slot * P + ep
    for T in range(3):
        mT = etbl == T
        idx_flat[T][ec[mT], tok_pos[mT]] = eloc[mT].astype(np.int16)

    def wrap(a):  # token-major -> wrapped [128, tokens//16]
        ncol = a.shape[1] // 16
        w = a.reshape(a.shape[0], ncol, 16).transpose(0, 2, 1)
        return np.ascontiguousarray(np.tile(w, (1, 8, 1)))

    idx_wrapped = [wrap(ix) for ix in idx_flat]

    # per-token deg_out in gather-output layout [128, tokens//128]
    nf = np.asarray(inputs["node_feats"], np.float32)
    tabs = []
    dtok = []
    for T in range(3):
        tb = np.zeros((TBL_ROWS[T], P), np.float32)
        nn = TBL_NNODES[T]
        tb[1:1 + nn, :IN_DIM] = nf[TBL_BASES[T]:TBL_BASES[T] + nn]
        tabs.append(tb)
        d = np.ones((NCORES, tok_total[T]), np.float32)
        mT = etbl == T
        d[ec[mT], tok_pos[mT]] = deg_out[src[mT]]
        dtok.append(np.ascontiguousarray(
            d.reshape(NCORES, tok_total[T] // P, P).transpose(0, 2, 1)))

    # one-hot proteins grouped 4/DMA: [PPC//4, 128, LCONV], p = g*4+s
    seq = np.asarray(inputs["protein_seq"]).reshape(NCORES, PPC, L)
    oh = np.zeros((NCORES, PPC, 32, LCONV), np.float32)
    iot = np.arange(VOCAB)[None, None, :, None]
    oh[:, :, :VOCAB, 1:1 + L] = (seq[:, :, None, :] == iot)
    oh = np.ascontiguousarray(
        oh.reshape(NCORES, PPC // 4, 4 * 32, LCONV))

    shared = {
        "tab0": tabs[0], "tab1": tabs[1], "tab2": tabs[2],
        "W_gc": np.asarray(inputs["W_gc"], np.float32),
        "b_gc": np.asarray(inputs["b_gc"], np.float32).reshape(HID, 1),
        "W_ro_in": np.asarray(inputs["W_ro_in"], np.float32),
        "b_ro_in": np.asarray(inputs["b_ro_in"], np.float32).reshape(HID, 1),
        "W_ro_out": np.asarray(inputs["W_ro_out"], np.float32),
        "b_ro_out": np.asarray(inputs["b_ro_out"], np.float32).reshape(HID, 1),
        "Wc1": np.asarray(inputs["Wc1"], np.float32),
        "bc1": np.asarray(inputs["bc1"], np.float32).reshape(HID, 1),
        "Wc2": np.asarray(inputs["Wc2"], np.float32),
        "bc2": np.asarray(inputs["bc2"], np.float32).reshape(HID, 1),
        "embedT": np.ascontiguousarray(
            np.asarray(inputs["embed"], np.float32).T),       # [HID, 25]
        "Wf1_r": np.ascontiguousarray(
            np.asarray(inputs["Wf1"], np.float32).reshape(2, HID, 2 * HID)),
        "bf1_r": np.ascontiguousarray(
            np.asarray(inputs["bf1"], np.float32).reshape(2, HID, 1)),
        "Wf2_r": np.ascontiguousarray(
            np.asarray(inputs["Wf2"], np.float32).reshape(2, HID, 1)),
        "bf2": np.asarray(inputs["bf2"], np.float32).reshape(1, 1),
    }
    for l in range(4):
        K = np.asarray(inputs["K%d" % (l + 1)], np.float32)  # [o, i, 3]
        shared["K%dT" % (l + 1)] = np.ascontiguousarray(
            K.transpose(1, 2, 0))                            # [i, 3, o]
        shared["cb%d" % (l + 1)] = np.asarray(
            inputs["cb%d" % (l + 1)], np.float32).reshape(-1, 1)

    percore = []
    for c in range(NCORES):
        percore.append({
            "deg_in_perm": np.ascontiguousarray(deg_in_perm[c]),
            "S": np.ascontiguousarray(S[c]),
            "onehot": np.ascontiguousarray(oh[c]),
            "ix0": idx_wrapped[0][c],
            "ix1": idx_wrapped[1][c],
            "ix2": idx_wrapped[2][c],
            "dtok0": dtok[0][c], "dtok1": dtok[1][c], "dtok2": dtok[2][c],
        })
    meta = dict(NT=NT, sched=sched, tok_total=tok_total)
    return shared, percore, meta


# --------------------------------------------------------------- device build
def _build(shared, meta):
    NT = meta["NT"]
    sched = meta["sched"]
    tok_total = meta["tok_total"]

    nc = bacc.Bacc("TRN2", target_bir_lowering=False, debug=False,
                   num_devices=NCORES, num_swdge_queues=4)
    f32, f32r, i16 = dt.float32, dt.float32r, dt.int16

    D = {k: nc.dram_tensor(k, list(v.shape), dt.from_np(v.dtype),
                           kind="ExternalInput")
         for k, v in shared.items()}
    D["deg_in_perm"] = nc.dram_tensor("deg_in_perm", [P, NT], f32,
                                      kind="ExternalInput")
    D["S"] = nc.dram_tensor("S", [NT, P, GPC], f32, kind="ExternalInput")
    D["onehot"] = nc.dram_tensor("onehot", [PPC // 4, P, LCONV], f32,
                                 kind="ExternalInput")
    for T in range(3):
        D["ix%d" % T] = nc.dram_tensor("ix%d" % T, [P, tok_total[T] // 16],
                                       i16, kind="ExternalInput")
    tabs = [D["tab%d" % T] for T in range(3)]
    for T in range(3):
        D["dtok%d" % T] = nc.dram_tensor("dtok%d" % T, [P, tok_total[T] // P],
                                         f32, kind="ExternalInput")
    out_d = nc.dram_tensor("out", [1, GPC], f32, kind="ExternalOutput")

    with tile.TileContext(nc) as tc, contextlib.ExitStack() as ctx:
        wp = ctx.enter_context(tc.tile_pool(name="wp", bufs=1))
        h0p = ctx.enter_context(tc.tile_pool(name="h0p", bufs=3))
        gp = ctx.enter_context(tc.tile_pool(name="gp", bufs=1))
        accp = ctx.enter_context(tc.tile_pool(name="accp", bufs=1))
        cvp = ctx.enter_context(tc.tile_pool(name="cvp", bufs=2))
        gnp = ctx.enter_context(tc.tile_pool(name="gnp", bufs=3))
        pcv = ctx.enter_context(tc.tile_pool(name="pcv", bufs=4, space="PSUM"))
        pgn = ctx.enter_context(tc.tile_pool(name="pgn", bufs=2, space="PSUM"))
        ps1 = ctx.enter_context(tc.tile_pool(name="ps1", bufs=1, space="PSUM"))

        # ---------------- setup: weights to SBUF
        def ld(name, shape, dtype=f32, src=None, tag=None):
            t = wp.tile(shape, dtype, tag=tag or name)
            ap = D[name][:] if src is None else src
            if dtype == f32r:
                ap = ap.bitcast(f32r)
            nc.sync.dma_start(out=t[:], in_=ap)
            return t

        W_gc = ld("W_gc", [IN_DIM, HID], f32r)
        b_gc = ld("b_gc", [HID, 1])
        W_ri = ld("W_ro_in", [HID, HID], f32r); b_ri = ld("b_ro_in", [HID, 1])
        W_ro = ld("W_ro_out", [HID, HID], f32r); b_ro = ld("b_ro_out", [HID, 1])
        Wc1 = ld("Wc1", [HID, HID], f32r); bc1 = ld("bc1", [HID, 1])
        Wc2 = ld("Wc2", [HID, HID], f32r); bc2 = ld("bc2", [HID, 1])
        Wf1 = ld("Wf1_r", [HID, 2, 2 * HID],
                 src=D["Wf1_r"][:].rearrange("k h m -> h k m"))
        bf1 = ld("bf1_r", [HID, 2, 1],
                 src=D["bf1_r"][:].rearrange("k h o -> h k o"))
        Wf2 = ld("Wf2_r", [HID, 2, 1],
                 src=D["Wf2_r"][:].rearrange("k h o -> h k o"))
        bf2 = ld("bf2", [1, 1])
        embT = ld("embedT", [HID, VOCAB], f32r)
        KT = [ld("K%dT" % (l + 1), [CHANNELS[l], 3, CHANNELS[l + 1]], f32r)
              for l in range(4)]
        cb = [ld("cb%d" % (l + 1), [CHANNELS[l + 1], 1]) for l in range(4)]
        Sg = ld("S", [P, NT, GPC], f32r,
                src=D["S"][:].rearrange("t p g -> p t g"))
        ixs = [ld("ix%d" % T, [P, tok_total[T] // 16], i16) for T in range(3)]
        dginp = ld("deg_in_perm", [P, NT])
        dts = [ld("dtok%d" % T, [P, tok_total[T] // P]) for T in range(3)]

        xb = []
        for l in range(3):
            pair = []
            for j in range(2):
                t = wp.tile([CHANNELS[l + 1], LCONV], f32r,
                            tag="xb%d_%d" % (l, j))
                nc.vector.memset(t[:, 0:1].bitcast(dt.float32), 0.0)
                nc.vector.memset(t[:, LCONV - 1:LCONV].bitcast(dt.float32),
                                 0.0)
                pair.append(t)
            xb.append(pair)

        ident = wp.tile([P, P], f32, tag="ident")
        make_identity(nc, ident[:])
        identr = wp.tile([P, P], f32r, tag="identr")
        nc.vector.tensor_copy(identr[:], ident[:])

        # rsqrt factors: w = sqrt(1/deg) per gather token / per dst lane
        for T in range(3):
            nc.vector.reciprocal(dts[T][:], dts[T][:])
            nc.scalar.sqrt(dts[T][:], dts[T][:])
        rdgi = wp.tile([P, NT], f32, tag="rdgi")
        nc.vector.reciprocal(rdgi[:], dginp[:])
        nc.scalar.sqrt(rdgi[:], rdgi[:])

        # M1rep[32s:32s+25, t, :] = embed @ K1_t^T replicated at 4 offsets
        M1rep = wp.tile([P, 3, CHANNELS[1]], f32r, tag="m1rep")
        for t in range(3):
            pm = ps1.tile([VOCAB, CHANNELS[1]], f32, space="PSUM", tag="ps1a")
            nc.tensor.matmul(pm[:], embT[:], KT[0][:, t, :], start=True,
                             stop=True)
            nc.scalar.copy(M1rep[:VOCAB, t, :], pm[:])
        for srow in range(1, 4):
            nc.sync.dma_start(out=M1rep[32 * srow:32 * srow + VOCAB, :, :],
                              in_=M1rep[:VOCAB, :, :])

        # ---------------- interleaved: conv proteins + gather groups
        acc = {}

        def emit_group(grp, after_protein=None):
            ohg = cvp.tile([P, LCONV], f32r, tag="ohg")
            nc.sync.dma_start(out=ohg[:], in_=D["onehot"][grp].bitcast(f32r))
            for srow in range(4):
                p = grp * 4 + srow
                b0 = 32 * srow
                xs = None
                for l in range(4):
                    cin, cout = CHANNELS[l], CHANNELS[l + 1]
                    for cchunk in range(2):
                        c0 = cchunk * 500
                        pps = pcv.tile([cout, 500], f32, space="PSUM",
                                       tag="cps")
                        for tap in range(3):
                            if l == 0:
                                lhsT = M1rep[b0:b0 + VOCAB, tap, :]
                                rhs = ohg[b0:b0 + VOCAB,
                                          c0 + tap:c0 + tap + 500]
                                tpos = (96, 0) if srow == 3 else None
                            else:
                                lhsT = KT[l][:, tap, :]
                                rhs = xs[:cin, c0 + tap:c0 + tap + 500]
                                tpos = None
                            nc.tensor.matmul(pps[:], lhsT, rhs,
                                             start=(tap == 0), stop=(tap == 2),
                                             tile_position=tpos)
                        if l < 3:
                            nc.scalar.activation(
                                xb[l][p % 2][:, 1 + c0:1 + c0 + 500],
                                pps[:], AF.Relu, bias=cb[l][:])
                        else:
                            nc.vector.reduce_max(
                                out=chunkmax[:, cchunk, p:p + 1],
                                in_=pps[:, :500], axis=AX.X)
                    if l < 3:
                        xs = xb[l][p % 2]
                if after_protein is not None:
                    after_protein(p)

        gjobs = []
        for T in range(3):
            off = 0
            for ins in sched[T]:
                gjobs.append((T, off, ins))
                off += sum(k * P for (_, k) in ins)

        def emit_gather(job, qn):
            T, off, ins = job
            ntok = sum(k * P for (_, k) in ins)
            g = gp.tile([P, ntok // P, P], f32, tag="g%d" % (qn % 6))
            nc.gpsimd.dma_gather(
                out_ap=g[:], in_ap=tabs[T][:],
                idxs_ap=ixs[T][:, off // 16:(off + ntok) // 16],
                num_idxs=ntok, num_idxs_reg=ntok, elem_size=P,
                single_packet=False, queue_num=qn % 4)
            blk0 = off // P
            nc.vector.tensor_tensor(
                out=g[:, :, :IN_DIM],
                in0=g[:, :, :IN_DIM],
                in1=dts[T][:, blk0:blk0 + ntok // P, None]
                    .to_broadcast([P, ntok // P, IN_DIM]),
                op=ALU.mult)
            boff = 0
            for (t, k) in ins:
                view = g[:, boff:boff + k, :IN_DIM].rearrange("p k d -> p d k")
                if t not in acc:
                    a = accp.tile([P, IN_DIM], f32, tag="acc%d" % t)
                    acc[t] = a
                    nc.vector.tensor_reduce(out=a[:], in_=view, axis=AX.X,
                                            op=ALU.add)
                else:
                    tmp = gp.tile([P, IN_DIM], f32, tag="rtmp")
                    nc.vector.tensor_reduce(out=tmp[:], in_=view, axis=AX.X,
                                            op=ALU.add)
                    nc.vector.tensor_add(out=acc[t][:], in0=acc[t][:],
                                         in1=tmp[:])
                boff += k

        pmax = wp.tile([P, PPC], f32, tag="pmax")
        chunkmax = wp.tile([P, 2, PPC], f32, tag="chunkmax")
        gq = list(gjobs)
        qst = [0]

        def drain(p):
            while gq and len(gq) > (PPC - 1 - p) * len(gjobs) // PPC:
                emit_gather(gq.pop(0), qst[0])
                qst[0] += 1

        for grp in range(PPC // 4):
            emit_group(grp, after_protein=drain)
        qn = qst[0]
        while gq:
            emit_gather(gq.pop(0), qn)
            qn += 1
        # pmax = relu(max(chunk maxes) + cb4)
        mxt = wp.tile([P, PPC], f32, tag="mxt")
        nc.vector.tensor_reduce(out=mxt[:],
                                in_=chunkmax[:].rearrange("p c q -> p q c"),
                                axis=AX.X, op=ALU.max)
        nc.scalar.activation(pmax[:], mxt[:], AF.Relu, bias=cb[3][:])
        # scale by rsqrt(deg_in)
        for t in range(NT):
            nc.vector.tensor_scalar_mul(acc[t][:], acc[t][:],
                                        rdgi[:, t:t + 1])

        # ---------------- GNN matmul chain (fp32)
        hg_ps = ps1.tile([GPC, HID], f32, space="PSUM", tag="hgps")
        for t in range(NT):
            tp = pgn.tile([IN_DIM, P], f32, space="PSUM", tag="gps")
            nc.tensor.transpose(tp[:], acc[t][:], ident[:])
            aggT = gnp.tile([IN_DIM, P], f32r, tag="aggT")
            nc.scalar.copy(aggT[:], tp[:])
            hps = pgn.tile([HID, P], f32, space="PSUM", tag="gps")
            nc.tensor.matmul(hps[:], W_gc[:], aggT[:], start=True, stop=True)
            h = gnp.tile([HID, P], f32r, tag="h")
            nc.scalar.activation(h[:], hps[:], AF.Relu, bias=b_gc[:])
            x1ps = pgn.tile([HID, P], f32, space="PSUM", tag="gps")
            nc.tensor.matmul(x1ps[:], W_ri[:], h[:], start=True, stop=True)
            x1 = gnp.tile([HID, P], f32r, tag="x1")
            nc.scalar.activation(x1[:], x1ps[:], AF.Identity, bias=b_ri[:])
            x2ps = pgn.tile([HID, P], f32, space="PSUM", tag="gps")
            nc.tensor.matmul(x2ps[:], W_ro[:], x1[:], start=True, stop=True)
            x2 = gnp.tile([HID, P], f32r, tag="x2")
            nc.scalar.activation(x2[:], x2ps[:], AF.Identity, bias=b_ro[:])
            x2t = pgn.tile([P, HID], f32r, space="PSUM", tag="gps")
            nc.tensor.transpose(x2t[:], x2[:], identr[:])
            x2n = gnp.tile([P, HID], f32r, tag="x2n")
            nc.scalar.copy(x2n[:], x2t[:])
            nc.tensor.matmul(hg_ps[:], Sg[:, t, :], x2n[:],
                             start=(t == 0), stop=(t == NT - 1),
                             skip_group_check=True)
        hgT = wp.tile([GPC, HID], f32, tag="hgT")
        nc.scalar.activation(hgT[:], hg_ps[:], AF.Relu)
        hgt_ps = pgn.tile([HID, GPC], f32, space="PSUM", tag="gps")
        nc.tensor.transpose(hgt_ps[:], hgT[:], ident[:GPC, :GPC])
        hg = wp.tile([HID, GPC], f32r, tag="hg")
        nc.scalar.copy(hg[:], hgt_ps[:])
        # compound FC
        c1ps = pgn.tile([HID, GPC], f32, space="PSUM", tag="gps")
        nc.tensor.matmul(c1ps[:], Wc1[:], hg[:], start=True, stop=True)
        cv1 = wp.tile([HID, GPC], f32r, tag="cv1")
        nc.scalar.activation(cv1[:], c1ps[:], AF.Relu, bias=bc1[:])
        c2ps = pgn.tile([HID, GPC], f32, space="PSUM", tag="gps")
        nc.tensor.matmul(c2ps[:], Wc2[:], cv1[:], start=True, stop=True)
        cv2 = wp.tile([HID, GPC], f32, tag="cv2")
        nc.scalar.activation(cv2[:], c2ps[:], AF.Relu, bias=bc2[:])
        # head: z = [cv2; pmax]
        zin = [cv2, pmax]
        z2 = []
        for mc in range(2):
            zps = pgn.tile([HID, GPC], f32, space="PSUM", tag="gps")
            for kc in range(2):
                nc.tensor.matmul(zps[:], Wf1[:, kc, mc * HID:(mc + 1) * HID],
                                 zin[kc][:, :GPC], start=(kc == 0),
                                 stop=(kc == 1))
            zt = wp.tile([HID, GPC], f32, tag="z2_%d" % mc)
            nc.scalar.activation(zt[:], zps[:], AF.Relu, bias=bf1[:, mc, :])
            z2.append(zt)
        ops = ps1.tile([1, GPC], f32, space="PSUM", tag="ps1a")
        for kc in range(2):
            nc.tensor.matmul(ops[:], Wf2[:, kc, :], z2[kc][:],
                             start=(kc == 0), stop=(kc == 1))
        ot = wp.tile([1, GPC], f32, tag="ot")
        nc.scalar.activation(ot[:], ops[:], AF.Sigmoid, bias=bf2[:1, :])
        nc.sync.dma_start(out=out_d[:], in_=ot[:])

    nc.compile()
    return nc


def kernel(**inputs):
    shared, percore, meta = _host_prep(inputs)
    nc = _build(shared, meta)
    in_maps = []
    for c in range(NCORES):
        m = dict(shared)
        m.update(percore[c])
        in_maps.append(m)
    res = run_bass_kernel_spmd(nc, in_maps, list(range(NCORES)))
    out = np.concatenate([res.results[c]["out"].reshape(GPC)
                          for c in range(NCORES)])
    return out.reshape(B, 1).astype(np.float32)


if __name__ == "__main__":
    sys.path.insert(0, "/root/problem")
    import jax
    import reference
    with jax.default_device(jax.devices("cpu")[0]):
        inputs = {k: np.asarray(v) for k, v in reference.setup_inputs().items()}
        exp = np.asarray(reference.reference(**inputs))
    got = kernel(**inputs)
    err = np.abs(got - exp).max()
    rel = err / max(np.abs(exp).max(), 1e-9)
    print("max abs err:", err, " rel:", rel)



# revision 11
# speedup vs baseline: 2.8972x; 2.8972x over previous
"""CPI_DGLLife kernel for 8 Trainium2 NeuronCores (SPMD).

GCN over a 65536-node graph + protein conv1d branch + CPI head.
Sharding: data-parallel over the 512-graph batch (64 graphs / core).

Aggregation: bf16 pair-row table (2 nodes / 512B row, prescaled by
rsqrt(deg_out)) gathered with exact edge tokens sorted by dst tile;
per-128-token blocks reduced onto dst lanes with one-hot Sel matmuls
(Sel built on-device via is_equal against an iota tile); self loops
added via an identity matmul of a contiguous per-core feature block.
"""
import sys
sys.path.insert(0, "/opt/trn_rl_repo")
import contextlib
import numpy as np

import concourse.bass as bass
import concourse.bacc as bacc
import concourse.tile as tile
from concourse import mybir
from concourse.bass_utils import run_bass_kernel_spmd
from concourse.masks import make_identity

dt = mybir.dt
AF = mybir.ActivationFunctionType
ALU = mybir.AluOpType
AX = mybir.AxisListType
BF16 = mybir.dt.np(dt.bfloat16)

P = 128
N, E, B, L = 65536, 262144, 512, 1000
IN_DIM, HID, VOCAB = 74, 128, 25
CHANNELS = [HID, 96, 128, IN_DIM, HID]
NCORES = 8
GPC = B // NCORES              # graphs per core = 64
PPC = GPC                      # proteins per core = 64
LCONV = 1002                   # 1000 + 2 guard cols
BPI = 32                       # gather blocks per dma_gather instruction
CB = 16                        # blocks per Sel chunk
KILL = 300.0                   # dst-lane code that matches no iota column
DEBUG_OUT = False              # extra pmax/cv2 outputs for error attribution


# ------------------------------------------------------------------ host prep
def _host_prep(inputs):
    graph_ids = np.asarray(inputs["graph_ids"]).astype(np.int64)
    src = np.asarray(inputs["edge_src"]).astype(np.int64)
    dst = np.asarray(inputs["edge_dst"]).astype(np.int64)
    deg_out = np.bincount(src, minlength=N).astype(np.float32) + 1.0
    deg_in = np.bincount(dst, minlength=N).astype(np.float32) + 1.0

    nf = np.asarray(inputs["node_feats"], np.float32)
    xs = nf / np.sqrt(deg_out)[:, None]              # prescaled [N, 74]
    tab = np.zeros((N // 2, 2 * P), BF16)
    tab[:, :IN_DIM] = xs[0::2]
    tab[:, P:P + IN_DIM] = xs[1::2]

    core_lo = np.searchsorted(graph_ids, np.arange(0, B + 1, GPC))
    ncore_nodes = core_lo[1:] - core_lo[:-1]
    NT = int(np.ceil(ncore_nodes.max() / P))
    NPAD = NT * P

    # per-core contiguous blocks: self features, rsqrt(deg_in), S matrix
    selfX = np.zeros((NCORES, P, NT, IN_DIM), BF16)
    rdgi = np.ones((NCORES, P, NT), np.float32)
    S = np.zeros((NCORES, P, NT, GPC), BF16)
    for c in range(NCORES):
        lo, hi = int(core_lo[c]), int(core_lo[c + 1])
        n = hi - lo
        v = np.arange(lo, hi)
        t, p = np.arange(n) // P, np.arange(n) % P
        selfX[c, p, t] = xs[v]
        rdgi[c, p, t] = 1.0 / np.sqrt(deg_in[v])
        S[c, p, t, graph_ids[v] - c * GPC] = 1.0

    cnt_g = np.bincount(graph_ids, minlength=B).astype(np.float32)
    assert cnt_g.max() < 256, "graph node count exceeds bf16 exact range"
    ncount = np.ascontiguousarray(cnt_g.reshape(NCORES, 1, GPC).astype(BF16))

    # edge tokens: sorted by (core, dst tile); per-tile block count is the
    # max over cores (SPMD uniform program)
    gid_d = graph_ids[dst]
    ec = gid_d // GPC
    pos = dst - core_lo[ec]
    et, ep = pos // P, pos % P
    cnt = np.zeros((NCORES, NT), np.int64)
    np.add.at(cnt, (ec, et), 1)
    nblk_t = np.ceil(cnt.max(axis=0) / P).astype(np.int64)     # [NT]
    blk0_t = np.concatenate([[0], np.cumsum(nblk_t)])
    NBLK = int(blk0_t[-1])
    NTOK = NBLK * P

    key = ec * NT + et
    order = np.argsort(key, kind="stable")
    ks = key[order]
    starts = np.r_[0, np.flatnonzero(np.diff(ks)) + 1]
    grp_len = np.diff(np.r_[starts, E])
    slot_sorted = np.arange(E) - np.repeat(starts, grp_len)
    slot = np.empty(E, np.int64)
    slot[order] = slot_sorted
    tok = blk0_t[et] * P + slot                       # token index per edge

    idx_flat = np.zeros((NCORES, NTOK), np.int16)
    idx_flat[ec, tok] = (src // 2).astype(np.int16)
    dl = np.full((NCORES, P, 2, NBLK), KILL, BF16)
    dl[ec, tok % P, src % 2, tok // P] = ep.astype(np.float32)

    def wrap(a):  # token-major -> wrapped [128, tokens//16]
        ncol = a.shape[1] // 16
        w = a.reshape(a.shape[0], ncol, 16).transpose(0, 2, 1)
        return np.ascontiguousarray(np.tile(w, (1, 8, 1)))

    idx_wrapped = wrap(idx_flat)

    # one-hot proteins grouped 4/DMA: [PPC//4, 128, LCONV] bf16
    seq = np.asarray(inputs["protein_seq"]).reshape(NCORES, PPC, L)
    oh = np.zeros((NCORES, PPC, 32, LCONV), BF16)
    iot = np.arange(VOCAB)[None, None, :, None]
    oh[:, :, :VOCAB, 1:1 + L] = (seq[:, :, None, :] == iot)
    oh = np.ascontiguousarray(oh.reshape(NCORES, PPC // 4, 4 * 32, LCONV))

    def b16(name):
        return np.asarray(inputs[name], np.float32).astype(BF16)

    shared = {
        "tab": tab,
        "W_gc": b16("W_gc"),
        "b_gc": np.asarray(inputs["b_gc"], np.float32).reshape(HID, 1),
        "W_ro_in": b16("W_ro_in"),
        "b_ro_in": np.asarray(inputs["b_ro_in"], np.float32).reshape(HID, 1),
        "W_ro_out": b16("W_ro_out"),
        "b_ro_row": np.ascontiguousarray(b16("b_ro_out").reshape(1, HID)),
        "Wc1": b16("Wc1"),
        "bc1": np.asarray(inputs["bc1"], np.float32).reshape(HID, 1),
        "Wc2": b16("Wc2"),
        "bc2": np.asarray(inputs["bc2"], np.float32).reshape(HID, 1),
        "embedT": np.ascontiguousarray(b16("embed").T),       # [HID, 25]
        "Wf1_r": np.ascontiguousarray(
            b16("Wf1").reshape(2, HID, 2 * HID)),
        "bf1_r": np.ascontiguousarray(
            np.asarray(inputs["bf1"], np.float32).reshape(2, HID, 1)),
        "Wf2_r": np.ascontiguousarray(b16("Wf2").reshape(2, HID, 1)),
        "bf2": np.asarray(inputs["bf2"], np.float32).reshape(1, 1),
    }
    for l in range(4):
        K = np.asarray(inputs["K%d" % (l + 1)], np.float32)  # [o, i, 3]
        shared["K%dT" % (l + 1)] = np.ascontiguousarray(
            K.transpose(1, 2, 0)).astype(BF16)               # [i, 3, o]
        shared["cb%d" % (l + 1)] = np.asarray(
            inputs["cb%d" % (l + 1)], np.float32).reshape(-1, 1)

    percore = []
    for c in range(NCORES):
        percore.append({
            "selfX": np.ascontiguousarray(selfX[c]),
            "rdgi": np.ascontiguousarray(rdgi[c]),
            "S": np.ascontiguousarray(S[c]),
            "ncount": ncount[c],
            "onehot": np.ascontiguousarray(oh[c]),
            "ix": idx_wrapped[c],
            "dl": np.ascontiguousarray(dl[c]),
        })
    meta = dict(NT=NT, NBLK=NBLK, NTOK=NTOK,
                nblk_t=nblk_t.tolist(), blk0_t=blk0_t.tolist())
    return shared, percore, meta


# --------------------------------------------------------------- device build
def _build(shared, meta):
    NT = meta["NT"]
    NBLK = meta["NBLK"]
    NTOK = meta["NTOK"]
    nblk_t = meta["nblk_t"]
    blk0_t = meta["blk0_t"]
    n_ginstr = (NBLK + BPI - 1) // BPI

    nc = bacc.Bacc("TRN2", target_bir_lowering=False, debug=False,
                   num_devices=NCORES, num_swdge_queues=4)
    f32, bf16, i16 = dt.float32, dt.bfloat16, dt.int16

    D = {k: nc.dram_tensor(k, list(v.shape), dt.from_np(v.dtype),
                           kind="ExternalInput")
         for k, v in shared.items()}
    D["selfX"] = nc.dram_tensor("selfX", [P, NT, IN_DIM], bf16,
                                kind="ExternalInput")
    D["rdgi"] = nc.dram_tensor("rdgi", [P, NT], f32, kind="ExternalInput")
    D["S"] = nc.dram_tensor("S", [P, NT, GPC], bf16, kind="ExternalInput")
    D["ncount"] = nc.dram_tensor("ncount", [1, GPC], bf16,
                                 kind="ExternalInput")
    D["onehot"] = nc.dram_tensor("onehot", [PPC // 4, P, LCONV], bf16,
                                 kind="ExternalInput")
    D["ix"] = nc.dram_tensor("ix", [P, NTOK // 16], i16, kind="ExternalInput")
    D["dl"] = nc.dram_tensor("dl", [P, 2, NBLK], bf16, kind="ExternalInput")
    out_d = nc.dram_tensor("out", [1, GPC], f32, kind="ExternalOutput")
    dbg_pmax = nc.dram_tensor("dbg_pmax", [P, PPC], f32,
                              kind="ExternalOutput") if DEBUG_OUT else None
    dbg_cv = nc.dram_tensor("dbg_cv", [HID, GPC], f32,
                            kind="ExternalOutput") if DEBUG_OUT else None

    with tile.TileContext(nc) as tc, contextlib.ExitStack() as ctx:
        wp = ctx.enter_context(tc.tile_pool(name="wp", bufs=1))
        gp = ctx.enter_context(tc.tile_pool(name="gp", bufs=1))
        selp = ctx.enter_context(tc.tile_pool(name="selp", bufs=1))
        accp = ctx.enter_context(tc.tile_pool(name="accp", bufs=3))
        cvp = ctx.enter_context(tc.tile_pool(name="cvp", bufs=2))
        gnp = ctx.enter_context(tc.tile_pool(name="gnp", bufs=3))
        pcv = ctx.enter_context(tc.tile_pool(name="pcv", bufs=3, space="PSUM"))
        pagg = ctx.enter_context(tc.tile_pool(name="pagg", bufs=2,
                                              space="PSUM"))
        pgn = ctx.enter_context(tc.tile_pool(name="pgn", bufs=2, space="PSUM"))
        phg = ctx.enter_context(tc.tile_pool(name="phg", bufs=1, space="PSUM"))

        # ---------------- setup: weights to SBUF
        def ld(name, shape, dtype=bf16, src=None, tag=None):
            t = wp.tile(shape, dtype, tag=tag or name)
            ap = D[name][:] if src is None else src
            nc.sync.dma_start(out=t[:], in_=ap)
            return t

        W_gc = ld("W_gc", [IN_DIM, HID])
        b_gc = ld("b_gc", [HID, 1], f32)
        W_ri = ld("W_ro_in", [HID, HID])
        b_ri = ld("b_ro_in", [HID, 1], f32)
        W_ro = ld("W_ro_out", [HID, HID])
        b_ro_row = ld("b_ro_row", [1, HID])
        Wc1 = ld("Wc1", [HID, HID]); bc1 = ld("bc1", [HID, 1], f32)
        Wc2 = ld("Wc2", [HID, HID]); bc2 = ld("bc2", [HID, 1], f32)
        Wf1 = ld("Wf1_r", [HID, 2, 2 * HID],
                 src=D["Wf1_r"][:].rearrange("k h m -> h k m"))
        bf1 = ld("bf1_r", [HID, 2, 1], f32,
                 src=D["bf1_r"][:].rearrange("k h o -> h k o"))
        Wf2 = ld("Wf2_r", [HID, 2, 1],
                 src=D["Wf2_r"][:].rearrange("k h o -> h k o"))
        bf2 = ld("bf2", [1, 1], f32)
        embT = ld("embedT", [HID, VOCAB])
        KT = [ld("K%dT" % (l + 1), [CHANNELS[l], 3, CHANNELS[l + 1]])
              for l in range(4)]
        cb = [ld("cb%d" % (l + 1), [CHANNELS[l + 1], 1], f32)
              for l in range(4)]
        Sg = ld("S", [P, NT, GPC])
        selfX = ld("selfX", [P, NT, IN_DIM])
        ncnt = ld("ncount", [1, GPC])
        ixt = ld("ix", [P, NTOK // 16], i16)
        dlt = ld("dl", [P, 2, NBLK])
        dgin = ld("rdgi", [P, NT], f32)

        xb = []
        for l in range(3):
            pair = []
            for j in range(2):
                t = wp.tile([CHANNELS[l + 1], LCONV], bf16,
                            tag="xb%d_%d" % (l, j))
                nc.vector.memset(t[:, 0:1], 0.0)
                nc.vector.memset(t[:, LCONV - 1:LCONV], 0.0)
                pair.append(t)
            xb.append(pair)

        ident = wp.tile([P, P], f32, tag="ident")
        make_identity(nc, ident[:])
        identb = wp.tile([P, P], bf16, tag="identb")
        nc.vector.tensor_copy(identb[:], ident[:])

        iota_big = wp.tile([P, CB, P], bf16, tag="iota_big")
        nc.gpsimd.iota(iota_big[:], [[0, CB], [1, P]], channel_multiplier=0,
                       allow_small_or_imprecise_dtypes=True)

        # host already sends rsqrt(deg_in) per (lane, tile)
        rdgi = dgin

        # M1rep[32s:32s+25, t, :] = embed @ K1_t^T replicated at 4 offsets
        M1rep = wp.tile([P, 3, CHANNELS[1]], bf16, tag="m1rep")
        for t in range(3):
            pm = pagg.tile([VOCAB, CHANNELS[1]], f32, space="PSUM",
                           tag="pagg")
            nc.tensor.matmul(pm[:], embT[:], KT[0][:, t, :], start=True,
                             stop=True)
            nc.scalar.copy(M1rep[:VOCAB, t, :], pm[:])
        for srow in range(1, 4):
            nc.sync.dma_start(out=M1rep[32 * srow:32 * srow + VOCAB, :, :],
                              in_=M1rep[:VOCAB, :, :])

        # ---------------- job helpers
        g_tiles = {}          # instr -> sbuf tile
        sel_tiles = {}        # (parity, chunk) -> sbuf tile
        hgst = [False]        # hg_ps accumulation started?
        hg_ps = phg.tile([GPC, HID], f32, space="PSUM", tag="hgps")

        def emit_gather(i):
            b0 = i * BPI
            nb = min(BPI, NBLK - b0)
            ntok = nb * P
            g = gp.tile([P, nb, 2 * P], bf16, tag="g%d" % (i % 3))
            off = b0 * P
            nc.gpsimd.dma_gather(
                out_ap=g[:], in_ap=D["tab"][:],
                idxs_ap=ixt[:, off // 16:(off + ntok) // 16],
                num_idxs=ntok, num_idxs_reg=ntok, elem_size=2 * P,
                single_packet=False, queue_num=i % 4)
            g_tiles[i] = g

        def get_sel(par, c):
            if (par, c) in sel_tiles:
                return sel_tiles[(par, c)]
            b0 = c * CB
            cbn = min(CB, NBLK - b0)
            s = selp.tile([P, cbn, P], bf16, tag="sel%d_%d" % (par, c % 3))
            nc.vector.tensor_tensor(
                out=s[:], in0=iota_big[:, :cbn, :],
                in1=dlt[:, par, b0:b0 + cbn, None].to_broadcast([P, cbn, P]),
                op=ALU.is_equal)
            sel_tiles[(par, c)] = s
            if (par, c - 3) in sel_tiles:
                del sel_tiles[(par, c - 3)]
            return s

        def emit_tile(t):
            # aggregate: acc[dst, 74] = sum_blocks Sel^T @ G + I @ selfX
            pa = pagg.tile([P, IN_DIM], f32, space="PSUM", tag="pagg")
            first = True
            for b in range(blk0_t[t], blk0_t[t] + nblk_t[t]):
                gi, gl = b // BPI, b % BPI
                ci, cl = b // CB, b % CB
                g = g_tiles[gi]
                for par in range(2):
                    s = get_sel(par, ci)
                    nc.tensor.matmul(
                        pa[:], s[:, cl, :],
                        g[:, gl, par * P:par * P + IN_DIM],
                        start=first, stop=False)
                    first = False
            nc.tensor.matmul(pa[:], identb[:], selfX[:, t, :],
                             start=first, stop=True)
            acc = accp.tile([P, IN_DIM], bf16, tag="acc")
            nc.vector.tensor_scalar_mul(acc[:], pa[:], rdgi[:, t:t + 1])
            # GNN chain for this tile
            tp = pgn.tile([IN_DIM, P], bf16, space="PSUM", tag="gps")
            nc.tensor.transpose(tp[:], acc[:], identb[:])
            aggT = gnp.tile([IN_DIM, P], bf16, tag="aggT")
            nc.scalar.copy(aggT[:], tp[:])
            hps = pgn.tile([HID, P], f32, space="PSUM", tag="gps")
            nc.tensor.matmul(hps[:], W_gc[:], aggT[:], start=True, stop=True)
            h = gnp.tile([HID, P], bf16, tag="h")
            nc.scalar.activation(h[:], hps[:], AF.Relu, bias=b_gc[:])
            x1ps = pgn.tile([HID, P], f32, space="PSUM", tag="gps")
            nc.tensor.matmul(x1ps[:], W_ri[:], h[:], start=True, stop=True)
            x1 = gnp.tile([HID, P], bf16, tag="x1")
            nc.vector.tensor_scalar_add(x1[:], x1ps[:], b_ri[:, 0:1])
            x2ps = pgn.tile([P, HID], f32, space="PSUM", tag="gps")
            nc.tensor.matmul(x2ps[:], x1[:], W_ro[:], start=True, stop=True)
            x2n = gnp.tile([P, HID], bf16, tag="x2n")
            nc.scalar.copy(x2n[:], x2ps[:])
            nc.tensor.matmul(hg_ps[:], Sg[:, t, :], x2n[:],
                             start=not hgst[0], stop=False,
                             skip_group_check=True)
            hgst[0] = True

        # ---------------- interleaved: conv proteins + gather/agg jobs
        # A gather instr's SBUF buffer rotates with depth 3 (tag i%3), so
        # every tile whose FIRST block falls in instr i-3 must be emitted
        # before instr i (tiles are in block order; a tile spans <=2 instrs).
        jobs = []
        done_tile = [0]

        def tiles_starting_below(blim):
            while done_tile[0] < NT and blk0_t[done_tile[0]] < blim:
                jobs.append(("t", done_tile[0]))
                done_tile[0] += 1

        for i in range(n_ginstr):
            if i >= 3:
                tiles_starting_below((i - 2) * BPI)
            jobs.append(("g", i))
        tiles_starting_below(NBLK + 1)

        def run_job(j):
            kind, a = j
            if kind == "g":
                emit_gather(a)
            else:
                emit_tile(a)

        chunkmax = wp.tile([P, 2, PPC], f32, tag="chunkmax")

        def emit_group(grp, after_protein=None):
            ohg = cvp.tile([P, LCONV], bf16, tag="ohg")
            nc.sync.dma_start(out=ohg[:], in_=D["onehot"][grp])
            for srow in range(4):
                p = grp * 4 + srow
                b0 = 32 * srow
                xs = None
                for l in range(4):
                    cin, cout = CHANNELS[l], CHANNELS[l + 1]
                    for cchunk in range(2):
                        c0 = cchunk * 500
                        pps = pcv.tile([cout, 500], f32, space="PSUM",
                                       tag="cps")
                        for tap in range(3):
                            if l == 0:
                                lhsT = M1rep[b0:b0 + VOCAB, tap, :]
                                rhs = ohg[b0:b0 + VOCAB,
                                          c0 + tap:c0 + tap + 500]
                                tpos = (96, 0) if srow == 3 else None
                            else:
                                lhsT = KT[l][:, tap, :]
                                rhs = xs[:cin, c0 + tap:c0 + tap + 500]
                                tpos = None
                            nc.tensor.matmul(pps[:], lhsT, rhs,
                                             start=(tap == 0), stop=(tap == 2),
                                             tile_position=tpos)
                        if l == 1:
                            nc.vector.tensor_scalar(
                                out=xb[l][p % 2][:, 1 + c0:1 + c0 + 500],
                                in0=pps[:], scalar1=cb[l][:, 0:1],
                                scalar2=0.0, op0=ALU.add, op1=ALU.max)
                        elif l < 3:
                            nc.scalar.activation(
                                xb[l][p % 2][:, 1 + c0:1 + c0 + 500],
                                pps[:], AF.Relu, bias=cb[l][:])
                        else:
                            nc.vector.reduce_max(
                                out=chunkmax[:, cchunk, p:p + 1],
                                in_=pps[:, :500], axis=AX.X)
                    if l < 3:
                        xs = xb[l][p % 2]
                if after_protein is not None:
                    after_protein(p)

        jq = list(jobs)

        def drain(p):
            while jq and len(jq) > (PPC - 1 - p) * len(jobs) // PPC:
                run_job(jq.pop(0))

        for grp in range(PPC // 4):
            emit_group(grp, after_protein=drain)
        while jq:
            run_job(jq.pop(0))

        # close hg accumulation: += ncount (x) b_ro
        nc.tensor.matmul(hg_ps[:], ncnt[:], b_ro_row[:],
                         start=False, stop=True, skip_group_check=True)

        # pmax = relu(max(chunk maxes) + cb4)
        pmax = wp.tile([P, PPC], bf16, tag="pmax")
        mxt = wp.tile([P, PPC], f32, tag="mxt")
        nc.vector.tensor_reduce(out=mxt[:],
                                in_=chunkmax[:].rearrange("p c q -> p q c"),
                                axis=AX.X, op=ALU.max)
        nc.scalar.activation(pmax[:], mxt[:], AF.Relu, bias=cb[3][:])
        if DEBUG_OUT:
            pmf = wp.tile([P, PPC], f32, tag="pmf")
            nc.vector.tensor_copy(pmf[:], pmax[:])
            nc.sync.dma_start(out=dbg_pmax[:], in_=pmf[:])

        # ---------------- readout + head
        hgT = wp.tile([GPC, HID], bf16, tag="hgT")
        nc.scalar.activation(hgT[:], hg_ps[:], AF.Relu)
        hgt_ps = pgn.tile([HID, GPC], bf16, space="PSUM", tag="gps")
        nc.tensor.transpose(hgt_ps[:], hgT[:], identb[:GPC, :GPC])
        hg = wp.tile([HID, GPC], bf16, tag="hg")
        nc.scalar.copy(hg[:], hgt_ps[:])
        c1ps = pgn.tile([HID, GPC], f32, space="PSUM", tag="gps")
        nc.tensor.matmul(c1ps[:], Wc1[:], hg[:], start=True, stop=True)
        cv1 = wp.tile([HID, GPC], bf16, tag="cv1")
        nc.scalar.activation(cv1[:], c1ps[:], AF.Relu, bias=bc1[:])
        c2ps = pgn.tile([HID, GPC], f32, space="PSUM", tag="gps")
        nc.tensor.matmul(c2ps[:], Wc2[:], cv1[:], start=True, stop=True)
        cv2 = wp.tile([HID, GPC], bf16, tag="cv2")
        nc.scalar.activation(cv2[:], c2ps[:], AF.Relu, bias=bc2[:])
        if DEBUG_OUT:
            cvf = wp.tile([HID, GPC], f32, tag="cvf")
            nc.vector.tensor_copy(cvf[:], cv2[:])
            nc.sync.dma_start(out=dbg_cv[:], in_=cvf[:])
        # head: z = [cv2; pmax]
        zin = [cv2, pmax]
        z2 = []
        for mc in range(2):
            zps = pgn.tile([HID, GPC], f32, space="PSUM", tag="gps")
            for kc in range(2):
                nc.tensor.matmul(zps[:], Wf1[:, kc, mc * HID:(mc + 1) * HID],
                                 zin[kc][:, :GPC], start=(kc == 0),
                                 stop=(kc == 1))
            zt = wp.tile([HID, GPC], bf16, tag="z2_%d" % mc)
            nc.scalar.activation(zt[:], zps[:], AF.Relu, bias=bf1[:, mc, :])
            z2.append(zt)
        ops = pagg.tile([1, GPC], f32, space="PSUM", tag="pagg")
        for kc in range(2):
            nc.tensor.matmul(ops[:], Wf2[:, kc, :], z2[kc][:],
                             start=(kc == 0), stop=(kc == 1))
        ot = wp.tile([1, GPC], f32, tag="ot")
        nc.scalar.activation(ot[:], ops[:], AF.Sigmoid, bias=bf2[:1, :])
        nc.sync.dma_start(out=out_d[:], in_=ot[:])

    nc.compile()
    return nc


def kernel(**inputs):
    shared, percore, meta = _host_prep(inputs)
    nc = _build(shared, meta)
    in_maps = []
    for c in range(NCORES):
        m = dict(shared)
        m.update(percore[c])
        in_maps.append(m)
    res = run_bass_kernel_spmd(nc, in_maps, list(range(NCORES)))
    out = np.concatenate([res.results[c]["out"].reshape(GPC)
                          for c in range(NCORES)])
    return out.reshape(B, 1).astype(np.float32)


if __name__ == "__main__":
    sys.path.insert(0, "/root/problem")
    import jax
    import reference
    with jax.default_device(jax.devices("cpu")[0]):
        inputs = {k: np.asarray(v) for k, v in reference.setup_inputs().items()}
        exp = np.asarray(reference.reference(**inputs))
    got = kernel(**inputs)
    err = np.abs(got - exp).max()
    rel = err / max(np.abs(exp).max(), 1e-9)
    print("max abs err:", err, " rel:", rel)


# revision 18
# speedup vs baseline: 3.0403x; 1.0494x over previous
"""CPI_DGLLife kernel for 8 Trainium2 NeuronCores (SPMD).

GCN over a 65536-node graph + protein conv1d branch + CPI head.
Sharding: data-parallel over the 512-graph batch (64 graphs / core).

Aggregation: bf16 pair-row table (2 nodes / 512B row, prescaled by
rsqrt(deg_out)) gathered with exact edge tokens sorted by dst tile;
per-128-token blocks reduced onto dst lanes with one-hot Sel matmuls
(Sel built on-device via is_equal against an iota tile); self loops
added via an identity matmul of a contiguous per-core feature block.
"""
import sys
sys.path.insert(0, "/opt/trn_rl_repo")
import contextlib
import numpy as np

import concourse.bass as bass
import concourse.bacc as bacc
import concourse.tile as tile
from concourse import mybir
from concourse.bass_utils import run_bass_kernel_spmd
from concourse.masks import make_identity

dt = mybir.dt
AF = mybir.ActivationFunctionType
ALU = mybir.AluOpType
AX = mybir.AxisListType
BF16 = mybir.dt.np(dt.bfloat16)

P = 128
N, E, B, L = 65536, 262144, 512, 1000
IN_DIM, HID, VOCAB = 74, 128, 25
CHANNELS = [HID, 96, 128, IN_DIM, HID]
NCORES = 8
GPC = B // NCORES              # graphs per core = 64
PPC = GPC                      # proteins per core = 64
LCONV = 1002                   # 1000 + 2 guard cols
BPI = 32                       # gather blocks per dma_gather instruction
CB = 16                        # blocks per Sel chunk
KILL = 300.0                   # dst-lane code that matches no iota column
DEBUG_OUT = False              # extra pmax/cv2 outputs for error attribution


# ------------------------------------------------------------------ host prep
def _host_prep(inputs):
    graph_ids = np.asarray(inputs["graph_ids"]).astype(np.int64)
    src = np.asarray(inputs["edge_src"]).astype(np.int64)
    dst = np.asarray(inputs["edge_dst"]).astype(np.int64)
    deg_out = np.bincount(src, minlength=N).astype(np.float32) + 1.0
    deg_in = np.bincount(dst, minlength=N).astype(np.float32) + 1.0

    nf = np.asarray(inputs["node_feats"], np.float32)
    xs = nf / np.sqrt(deg_out)[:, None]              # prescaled [N, 74]
    tab = np.zeros((N // 2, 2 * P), BF16)
    tab[:, :IN_DIM] = xs[0::2]
    tab[:, P:P + IN_DIM] = xs[1::2]

    core_lo = np.searchsorted(graph_ids, np.arange(0, B + 1, GPC))
    ncore_nodes = core_lo[1:] - core_lo[:-1]
    NT = int(np.ceil(ncore_nodes.max() / P))
    NPAD = NT * P

    # per-core contiguous blocks: self features, rsqrt(deg_in), S matrix
    selfX = np.zeros((NCORES, P, NT, IN_DIM), BF16)
    rdgi = np.ones((NCORES, P, NT), np.float32)
    S = np.zeros((NCORES, P, NT, GPC), BF16)
    for c in range(NCORES):
        lo, hi = int(core_lo[c]), int(core_lo[c + 1])
        n = hi - lo
        v = np.arange(lo, hi)
        t, p = np.arange(n) // P, np.arange(n) % P
        selfX[c, p, t] = xs[v]
        rdgi[c, p, t] = 1.0 / np.sqrt(deg_in[v])
        S[c, p, t, graph_ids[v] - c * GPC] = 1.0

    cnt_g = np.bincount(graph_ids, minlength=B).astype(np.float32)
    assert cnt_g.max() < 256, "graph node count exceeds bf16 exact range"
    ncount = np.ascontiguousarray(cnt_g.reshape(NCORES, 1, GPC).astype(BF16))

    # edge tokens: sorted by (core, dst tile); per-tile block count is the
    # max over cores (SPMD uniform program)
    gid_d = graph_ids[dst]
    ec = gid_d // GPC
    pos = dst - core_lo[ec]
    et, ep = pos // P, pos % P
    cnt = np.zeros((NCORES, NT), np.int64)
    np.add.at(cnt, (ec, et), 1)
    nblk_t = np.ceil(cnt.max(axis=0) / P).astype(np.int64)     # [NT]
    blk0_t = np.concatenate([[0], np.cumsum(nblk_t)])
    NBLK = int(blk0_t[-1])
    NTOK = NBLK * P

    key = ec * NT + et
    order = np.argsort(key, kind="stable")
    ks = key[order]
    starts = np.r_[0, np.flatnonzero(np.diff(ks)) + 1]
    grp_len = np.diff(np.r_[starts, E])
    slot_sorted = np.arange(E) - np.repeat(starts, grp_len)
    slot = np.empty(E, np.int64)
    slot[order] = slot_sorted
    tok = blk0_t[et] * P + slot                       # token index per edge

    idx_flat = np.zeros((NCORES, NTOK), np.int16)
    idx_flat[ec, tok] = (src // 2).astype(np.int16)
    dl = np.full((NCORES, P, 2, NBLK), KILL, BF16)
    dl[ec, tok % P, src % 2, tok // P] = ep.astype(np.float32)

    def wrap(a):  # token-major -> wrapped [128, tokens//16]
        ncol = a.shape[1] // 16
        w = a.reshape(a.shape[0], ncol, 16).transpose(0, 2, 1)
        return np.ascontiguousarray(np.tile(w, (1, 8, 1)))

    idx_wrapped = wrap(idx_flat)

    # one-hot proteins grouped 4/DMA: [PPC//4, 128, LCONV] bf16
    seq = np.asarray(inputs["protein_seq"]).reshape(NCORES, PPC, L)
    oh = np.zeros((NCORES, PPC, 32, LCONV), BF16)
    iot = np.arange(VOCAB)[None, None, :, None]
    oh[:, :, :VOCAB, 1:1 + L] = (seq[:, :, None, :] == iot)
    oh = np.ascontiguousarray(oh.reshape(NCORES, PPC // 4, 4 * 32, LCONV))

    def b16(name):
        return np.asarray(inputs[name], np.float32).astype(BF16)

    shared = {
        "tab": tab,
        "W_gc": b16("W_gc"),
        "b_gc": np.asarray(inputs["b_gc"], np.float32).reshape(HID, 1),
        "W_ro_in": b16("W_ro_in"),
        "b_ro_in": np.asarray(inputs["b_ro_in"], np.float32).reshape(HID, 1),
        "W_ro_out": b16("W_ro_out"),
        "b_ro_row": np.ascontiguousarray(b16("b_ro_out").reshape(1, HID)),
        "Wc1": b16("Wc1"),
        "bc1": np.asarray(inputs["bc1"], np.float32).reshape(HID, 1),
        "Wc2": b16("Wc2"),
        "bc2": np.asarray(inputs["bc2"], np.float32).reshape(HID, 1),
        "embedT": np.ascontiguousarray(b16("embed").T),       # [HID, 25]
        "Wf1_r": np.ascontiguousarray(
            b16("Wf1").reshape(2, HID, 2 * HID)),
        "bf1_r": np.ascontiguousarray(
            np.asarray(inputs["bf1"], np.float32).reshape(2, HID, 1)),
        "Wf2_r": np.ascontiguousarray(b16("Wf2").reshape(2, HID, 1)),
        "bf2": np.asarray(inputs["bf2"], np.float32).reshape(1, 1),
    }
    for l in range(4):
        K = np.asarray(inputs["K%d" % (l + 1)], np.float32)  # [o, i, 3]
        shared["K%dT" % (l + 1)] = np.ascontiguousarray(
            K.transpose(1, 2, 0)).astype(BF16)               # [i, 3, o]
        shared["cb%d" % (l + 1)] = np.asarray(
            inputs["cb%d" % (l + 1)], np.float32).reshape(-1, 1)

    percore = []
    for c in range(NCORES):
        percore.append({
            "selfX": np.ascontiguousarray(selfX[c]),
            "rdgi": np.ascontiguousarray(rdgi[c]),
            "S": np.ascontiguousarray(S[c]),
            "ncount": ncount[c],
            "onehot": np.ascontiguousarray(oh[c]),
            "ix": idx_wrapped[c],
            "dl": np.ascontiguousarray(dl[c]),
        })
    meta = dict(NT=NT, NBLK=NBLK, NTOK=NTOK,
                nblk_t=nblk_t.tolist(), blk0_t=blk0_t.tolist())
    return shared, percore, meta


# --------------------------------------------------------------- device build
def _build(shared, meta):
    NT = meta["NT"]
    NBLK = meta["NBLK"]
    NTOK = meta["NTOK"]
    nblk_t = meta["nblk_t"]
    blk0_t = meta["blk0_t"]
    n_ginstr = (NBLK + BPI - 1) // BPI

    nc = bacc.Bacc("TRN2", target_bir_lowering=False, debug=False,
                   num_devices=NCORES, num_swdge_queues=4)
    f32, bf16, i16 = dt.float32, dt.bfloat16, dt.int16

    D = {k: nc.dram_tensor(k, list(v.shape), dt.from_np(v.dtype),
                           kind="ExternalInput")
         for k, v in shared.items()}
    D["selfX"] = nc.dram_tensor("selfX", [P, NT, IN_DIM], bf16,
                                kind="ExternalInput")
    D["rdgi"] = nc.dram_tensor("rdgi", [P, NT], f32, kind="ExternalInput")
    D["S"] = nc.dram_tensor("S", [P, NT, GPC], bf16, kind="ExternalInput")
    D["ncount"] = nc.dram_tensor("ncount", [1, GPC], bf16,
                                 kind="ExternalInput")
    D["onehot"] = nc.dram_tensor("onehot", [PPC // 4, P, LCONV], bf16,
                                 kind="ExternalInput")
    D["ix"] = nc.dram_tensor("ix", [P, NTOK // 16], i16, kind="ExternalInput")
    D["dl"] = nc.dram_tensor("dl", [P, 2, NBLK], bf16, kind="ExternalInput")
    out_d = nc.dram_tensor("out", [1, GPC], f32, kind="ExternalOutput")
    dbg_pmax = nc.dram_tensor("dbg_pmax", [P, PPC], f32,
                              kind="ExternalOutput") if DEBUG_OUT else None
    dbg_cv = nc.dram_tensor("dbg_cv", [HID, GPC], f32,
                            kind="ExternalOutput") if DEBUG_OUT else None

    with tile.TileContext(nc) as tc, contextlib.ExitStack() as ctx:
        wp = ctx.enter_context(tc.tile_pool(name="wp", bufs=1))
        gp = ctx.enter_context(tc.tile_pool(name="gp", bufs=1))
        selp = ctx.enter_context(tc.tile_pool(name="selp", bufs=1))
        accp = ctx.enter_context(tc.tile_pool(name="accp", bufs=3))
        cvp = ctx.enter_context(tc.tile_pool(name="cvp", bufs=2))
        gnp = ctx.enter_context(tc.tile_pool(name="gnp", bufs=3))
        pcv = ctx.enter_context(tc.tile_pool(name="pcv", bufs=5, space="PSUM"))
        pgn = ctx.enter_context(tc.tile_pool(name="pgn", bufs=2, space="PSUM"))
        phg = ctx.enter_context(tc.tile_pool(name="phg", bufs=1, space="PSUM"))

        # ---------------- setup: weights to SBUF
        def ld(name, shape, dtype=bf16, src=None, tag=None):
            t = wp.tile(shape, dtype, tag=tag or name)
            ap = D[name][:] if src is None else src
            nc.sync.dma_start(out=t[:], in_=ap)
            return t

        W_gc = ld("W_gc", [IN_DIM, HID])
        b_gc = ld("b_gc", [HID, 1], f32)
        W_ri = ld("W_ro_in", [HID, HID])
        b_ri = ld("b_ro_in", [HID, 1], f32)
        W_ro = ld("W_ro_out", [HID, HID])
        b_ro_row = ld("b_ro_row", [1, HID])
        Wc1 = ld("Wc1", [HID, HID]); bc1 = ld("bc1", [HID, 1], f32)
        Wc2 = ld("Wc2", [HID, HID]); bc2 = ld("bc2", [HID, 1], f32)
        Wf1 = ld("Wf1_r", [HID, 2, 2 * HID],
                 src=D["Wf1_r"][:].rearrange("k h m -> h k m"))
        bf1 = ld("bf1_r", [HID, 2, 1], f32,
                 src=D["bf1_r"][:].rearrange("k h o -> h k o"))
        Wf2 = ld("Wf2_r", [HID, 2, 1],
                 src=D["Wf2_r"][:].rearrange("k h o -> h k o"))
        bf2 = ld("bf2", [1, 1], f32)
        embT = ld("embedT", [HID, VOCAB])
        KT = [ld("K%dT" % (l + 1), [CHANNELS[l], 3, CHANNELS[l + 1]])
              for l in range(4)]
        cb = [ld("cb%d" % (l + 1), [CHANNELS[l + 1], 1], f32)
              for l in range(4)]
        Sg = ld("S", [P, NT, GPC])
        selfX = ld("selfX", [P, NT, IN_DIM])
        ncnt = ld("ncount", [1, GPC])
        ixt = ld("ix", [P, NTOK // 16], i16)
        dlt = ld("dl", [P, 2, NBLK])
        dgin = ld("rdgi", [P, NT], f32)

        xb = []
        for l in range(3):
            pair = []
            for j in range(2):
                t = wp.tile([CHANNELS[l + 1], LCONV], bf16,
                            tag="xb%d_%d" % (l, j))
                nc.vector.memset(t[:, 0:1], 0.0)
                nc.vector.memset(t[:, LCONV - 1:LCONV], 0.0)
                pair.append(t)
            xb.append(pair)

        ident = wp.tile([P, P], f32, tag="ident")
        make_identity(nc, ident[:])
        identb = wp.tile([P, P], bf16, tag="identb")
        nc.vector.tensor_copy(identb[:], ident[:])

        iota_big = wp.tile([P, CB, P], bf16, tag="iota_big")
        nc.gpsimd.iota(iota_big[:], [[0, CB], [1, P]], channel_multiplier=0,
                       allow_small_or_imprecise_dtypes=True)

        # host already sends rsqrt(deg_in) per (lane, tile)
        rdgi = dgin

        # M1rep[32s:32s+25, t, :] = embed @ K1_t^T replicated at 4 offsets
        M1rep = wp.tile([P, 3, CHANNELS[1]], bf16, tag="m1rep")
        for t in range(3):
            pm = pgn.tile([VOCAB, CHANNELS[1]], f32, space="PSUM",
                          tag="gps")
            nc.tensor.matmul(pm[:], embT[:], KT[0][:, t, :], start=True,
                             stop=True)
            nc.scalar.copy(M1rep[:VOCAB, t, :], pm[:])
        for srow in range(1, 4):
            nc.sync.dma_start(out=M1rep[32 * srow:32 * srow + VOCAB, :, :],
                              in_=M1rep[:VOCAB, :, :])

        # ---------------- job helpers
        g_tiles = {}          # instr -> sbuf tile
        sel_tiles = {}        # (parity, chunk) -> sbuf tile
        hgst = [False]        # hg_ps accumulation started?
        hg_ps = phg.tile([GPC, HID], f32, space="PSUM", tag="hgps")

        def emit_gather(i):
            b0 = i * BPI
            nb = min(BPI, NBLK - b0)
            ntok = nb * P
            g = gp.tile([P, nb, 2 * P], bf16, tag="g%d" % (i % 3))
            off = b0 * P
            nc.gpsimd.dma_gather(
                out_ap=g[:], in_ap=D["tab"][:],
                idxs_ap=ixt[:, off // 16:(off + ntok) // 16],
                num_idxs=ntok, num_idxs_reg=ntok, elem_size=2 * P,
                single_packet=False, queue_num=i % 4)
            g_tiles[i] = g
            # prebuild the Sel chunks this instr's blocks will need
            for c in range(b0 // CB, (b0 + nb + CB - 1) // CB):
                for par in range(2):
                    if (par, c) in sel_tiles:
                        continue
                    c0 = c * CB
                    cbn = min(CB, NBLK - c0)
                    s = selp.tile([P, cbn, P], bf16,
                                  tag="sel%d_%d" % (par, c % 6))
                    nc.vector.tensor_tensor(
                        out=s[:], in0=iota_big[:, :cbn, :],
                        in1=dlt[:, par, c0:c0 + cbn, None]
                            .to_broadcast([P, cbn, P]),
                        op=ALU.is_equal)
                    sel_tiles[(par, c)] = s

        def emit_tile_agg(t):
            # aggregate: acc[dst, 74] = sum_blocks Sel^T @ G + I @ selfX
            pa = pgn.tile([P, IN_DIM], f32, space="PSUM", tag="gps")
            first = True
            for b in range(blk0_t[t], blk0_t[t] + nblk_t[t]):
                gi, gl = b // BPI, b % BPI
                ci, cl = b // CB, b % CB
                g = g_tiles[gi]
                for par in range(2):
                    s = sel_tiles[(par, ci)]
                    nc.tensor.matmul(
                        pa[:], s[:, cl, :],
                        g[:, gl, par * P:par * P + IN_DIM],
                        start=first, stop=False)
                    first = False
            nc.tensor.matmul(pa[:], identb[:], selfX[:, t, :],
                             start=first, stop=True)
            acc = accp.tile([P, IN_DIM], bf16, tag="acc")
            nc.vector.tensor_scalar_mul(acc[:], pa[:], rdgi[:, t:t + 1])
            tp = pgn.tile([IN_DIM, P], bf16, space="PSUM", tag="gps")
            nc.tensor.transpose(tp[:], acc[:], identb[:])
            aggT = gnp.tile([IN_DIM, P], bf16, tag="aggT")
            nc.scalar.copy(aggT[:], tp[:])
            return aggT

        def emit_tile_gnn(t, aggT):
            hps = pgn.tile([HID, P], f32, space="PSUM", tag="gps")
            nc.tensor.matmul(hps[:], W_gc[:], aggT[:], start=True, stop=True)
            h = gnp.tile([HID, P], bf16, tag="h")
            nc.scalar.activation(h[:], hps[:], AF.Relu, bias=b_gc[:])
            x1ps = pgn.tile([HID, P], f32, space="PSUM", tag="gps")
            nc.tensor.matmul(x1ps[:], W_ri[:], h[:], start=True, stop=True)
            x1 = gnp.tile([HID, P], bf16, tag="x1")
            nc.vector.tensor_scalar_add(x1[:], x1ps[:], b_ri[:, 0:1])
            x2ps = pgn.tile([P, HID], f32, space="PSUM", tag="gps")
            nc.tensor.matmul(x2ps[:], x1[:], W_ro[:], start=True, stop=True)
            x2n = gnp.tile([P, HID], bf16, tag="x2n")
            nc.scalar.copy(x2n[:], x2ps[:])
            nc.tensor.matmul(hg_ps[:], Sg[:, t, :], x2n[:],
                             start=not hgst[0], stop=False,
                             skip_group_check=True)
            hgst[0] = True

        # ---------------- interleaved: conv proteins + gather/agg jobs
        # A gather instr's SBUF buffer rotates with depth 3 (tag i%3), so
        # every tile whose FIRST block falls in instr i-3 must be emitted
        # before instr i (tiles are in block order; a tile spans <=2 instrs).
        # Each tile splits into an agg job ("ta") and a gnn job ("tg"); the
        # gnn job is delayed one tile so the aggT handoff latency is hidden
        # behind the next tile's agg matmuls.
        jobs = []
        done_tile = [0]

        def tiles_starting_below(blim):
            while done_tile[0] < NT and blk0_t[done_tile[0]] < blim:
                t = done_tile[0]
                jobs.append(("ta", t))
                if t > 0:
                    jobs.append(("tg", t - 1))
                done_tile[0] += 1

        for i in range(n_ginstr):
            if i >= 3:
                tiles_starting_below((i - 2) * BPI)
            jobs.append(("g", i))
        tiles_starting_below(NBLK + 1)
        jobs.append(("tg", NT - 1))

        aggT_store = {}

        def run_job(j):
            kind, a = j
            if kind == "g":
                emit_gather(a)
            elif kind == "ta":
                aggT_store[a] = emit_tile_agg(a)
            else:
                emit_tile_gnn(a, aggT_store.pop(a))

        chunkmax = wp.tile([P, 2, PPC], f32, tag="chunkmax")

        def emit_group(grp, after_pair=None):
            # layer-interleaved protein pairs: the PE streams protein p+1's
            # layer while p's activation drains, removing the act-latency
            # stall between layers.
            ohg = cvp.tile([P, LCONV], bf16, tag="ohg")
            nc.sync.dma_start(out=ohg[:], in_=D["onehot"][grp])
            for pair in range(2):
                for l in range(4):
                    cin, cout = CHANNELS[l], CHANNELS[l + 1]
                    for srow in (2 * pair, 2 * pair + 1):
                        p = grp * 4 + srow
                        b0 = 32 * srow
                        xs = xb[l - 1][p % 2] if l > 0 else None
                        for cchunk in range(2):
                            c0 = cchunk * 500
                            pps = pcv.tile([cout, 500], f32, space="PSUM",
                                           tag="cps")
                            for tap in range(3):
                                if l == 0:
                                    lhsT = M1rep[b0:b0 + VOCAB, tap, :]
                                    rhs = ohg[b0:b0 + VOCAB,
                                              c0 + tap:c0 + tap + 500]
                                    tpos = (96, 0) if srow == 3 else None
                                else:
                                    lhsT = KT[l][:, tap, :]
                                    rhs = xs[:cin, c0 + tap:c0 + tap + 500]
                                    tpos = None
                                nc.tensor.matmul(pps[:], lhsT, rhs,
                                                 start=(tap == 0),
                                                 stop=(tap == 2),
                                                 tile_position=tpos)
                            if l == 1:
                                nc.vector.tensor_scalar(
                                    out=xb[l][p % 2][:, 1 + c0:1 + c0 + 500],
                                    in0=pps[:], scalar1=cb[l][:, 0:1],
                                    scalar2=0.0, op0=ALU.add, op1=ALU.max)
                            elif l < 3:
                                nc.scalar.activation(
                                    xb[l][p % 2][:, 1 + c0:1 + c0 + 500],
                                    pps[:], AF.Relu, bias=cb[l][:])
                            else:
                                nc.vector.reduce_max(
                                    out=chunkmax[:, cchunk, p:p + 1],
                                    in_=pps[:, :500], axis=AX.X)
                if after_pair is not None:
                    after_pair(grp * 4 + 2 * pair + 1)

        jq = list(jobs)

        def drain(p):
            while jq and len(jq) > (PPC - 1 - p) * len(jobs) // PPC:
                run_job(jq.pop(0))

        for grp in range(PPC // 4):
            emit_group(grp, after_pair=drain)
        while jq:
            run_job(jq.pop(0))

        # close hg accumulation: += ncount (x) b_ro
        nc.tensor.matmul(hg_ps[:], ncnt[:], b_ro_row[:],
                         start=False, stop=True, skip_group_check=True)

        # pmax = relu(max(chunk maxes) + cb4)
        pmax = wp.tile([P, PPC], bf16, tag="pmax")
        mxt = wp.tile([P, PPC], f32, tag="mxt")
        nc.vector.tensor_reduce(out=mxt[:],
                                in_=chunkmax[:].rearrange("p c q -> p q c"),
                                axis=AX.X, op=ALU.max)
        nc.scalar.activation(pmax[:], mxt[:], AF.Relu, bias=cb[3][:])
        if DEBUG_OUT:
            pmf = wp.tile([P, PPC], f32, tag="pmf")
            nc.vector.tensor_copy(pmf[:], pmax[:])
            nc.sync.dma_start(out=dbg_pmax[:], in_=pmf[:])

        # ---------------- readout + head
        hgT = wp.tile([GPC, HID], bf16, tag="hgT")
        nc.scalar.activation(hgT[:], hg_ps[:], AF.Relu)
        hgt_ps = pgn.tile([HID, GPC], bf16, space="PSUM", tag="gps")
        nc.tensor.transpose(hgt_ps[:], hgT[:], identb[:GPC, :GPC])
        hg = wp.tile([HID, GPC], bf16, tag="hg")
        nc.scalar.copy(hg[:], hgt_ps[:])
        c1ps = pgn.tile([HID, GPC], f32, space="PSUM", tag="gps")
        nc.tensor.matmul(c1ps[:], Wc1[:], hg[:], start=True, stop=True)
        cv1 = wp.tile([HID, GPC], bf16, tag="cv1")
        nc.scalar.activation(cv1[:], c1ps[:], AF.Relu, bias=bc1[:])
        c2ps = pgn.tile([HID, GPC], f32, space="PSUM", tag="gps")
        nc.tensor.matmul(c2ps[:], Wc2[:], cv1[:], start=True, stop=True)
        cv2 = wp.tile([HID, GPC], bf16, tag="cv2")
        nc.scalar.activation(cv2[:], c2ps[:], AF.Relu, bias=bc2[:])
        if DEBUG_OUT:
            cvf = wp.tile([HID, GPC], f32, tag="cvf")
            nc.vector.tensor_copy(cvf[:], cv2[:])
            nc.sync.dma_start(out=dbg_cv[:], in_=cvf[:])
        # head: z = [cv2; pmax]
        zin = [cv2, pmax]
        z2 = []
        for mc in range(2):
            zps = pgn.tile([HID, GPC], f32, space="PSUM", tag="gps")
            for kc in range(2):
                nc.tensor.matmul(zps[:], Wf1[:, kc, mc * HID:(mc + 1) * HID],
                                 zin[kc][:, :GPC], start=(kc == 0),
                                 stop=(kc == 1))
            zt = wp.tile([HID, GPC], bf16, tag="z2_%d" % mc)
            nc.scalar.activation(zt[:], zps[:], AF.Relu, bias=bf1[:, mc, :])
            z2.append(zt)
        ops = pgn.tile([1, GPC], f32, space="PSUM", tag="gps")
        for kc in range(2):
            nc.tensor.matmul(ops[:], Wf2[:, kc, :], z2[kc][:],
                             start=(kc == 0), stop=(kc == 1))
        ot = wp.tile([1, GPC], f32, tag="ot")
        nc.scalar.activation(ot[:], ops[:], AF.Sigmoid, bias=bf2[:1, :])
        nc.sync.dma_start(out=out_d[:], in_=ot[:])

    nc.compile()
    return nc


def kernel(**inputs):
    shared, percore, meta = _host_prep(inputs)
    nc = _build(shared, meta)
    in_maps = []
    for c in range(NCORES):
        m = dict(shared)
        m.update(percore[c])
        in_maps.append(m)
    res = run_bass_kernel_spmd(nc, in_maps, list(range(NCORES)))
    out = np.concatenate([res.results[c]["out"].reshape(GPC)
                          for c in range(NCORES)])
    return out.reshape(B, 1).astype(np.float32)


if __name__ == "__main__":
    sys.path.insert(0, "/root/problem")
    import jax
    import reference
    with jax.default_device(jax.devices("cpu")[0]):
        inputs = {k: np.asarray(v) for k, v in reference.setup_inputs().items()}
        exp = np.asarray(reference.reference(**inputs))
    got = kernel(**inputs)
    err = np.abs(got - exp).max()
    rel = err / max(np.abs(exp).max(), 1e-9)
    print("max abs err:", err, " rel:", rel)


# revision 22
# speedup vs baseline: 3.1704x; 1.0428x over previous
"""CPI_DGLLife kernel for 8 Trainium2 NeuronCores (SPMD).

GCN over a 65536-node graph + protein conv1d branch + CPI head.
Sharding: data-parallel over the 512-graph batch (64 graphs / core).

Aggregation: bf16 pair-row table (2 nodes / 512B row, prescaled by
rsqrt(deg_out)) gathered with exact edge tokens sorted by dst tile;
per-128-token blocks reduced onto dst lanes with one-hot Sel matmuls
(Sel built on-device via is_equal against an iota tile); self loops
added via an identity matmul of a contiguous per-core feature block.
"""
import sys
sys.path.insert(0, "/opt/trn_rl_repo")
import contextlib
import numpy as np

import concourse.bass as bass
import concourse.bacc as bacc
import concourse.tile as tile
from concourse import mybir
from concourse.bass_utils import run_bass_kernel_spmd
from concourse.masks import make_identity

dt = mybir.dt
AF = mybir.ActivationFunctionType
ALU = mybir.AluOpType
AX = mybir.AxisListType
BF16 = mybir.dt.np(dt.bfloat16)

P = 128
N, E, B, L = 65536, 262144, 512, 1000
IN_DIM, HID, VOCAB = 74, 128, 25
CHANNELS = [HID, 96, 128, IN_DIM, HID]
NCORES = 8
GPC = B // NCORES              # graphs per core = 64
PPC = GPC                      # proteins per core = 64
LCONV = 1002                   # 1000 + 2 guard cols
BPI = 32                       # gather blocks per dma_gather instruction
CB = 16                        # blocks per Sel chunk
KILL = 300.0                   # dst-lane code that matches no iota column
DEBUG_OUT = False              # extra pmax/cv2 outputs for error attribution


# ------------------------------------------------------------------ host prep
def _host_prep(inputs):
    graph_ids = np.asarray(inputs["graph_ids"]).astype(np.int64)
    src = np.asarray(inputs["edge_src"]).astype(np.int64)
    dst = np.asarray(inputs["edge_dst"]).astype(np.int64)
    deg_out = np.bincount(src, minlength=N).astype(np.float32) + 1.0
    deg_in = np.bincount(dst, minlength=N).astype(np.float32) + 1.0

    nf = np.asarray(inputs["node_feats"], np.float32)
    xs = nf / np.sqrt(deg_out)[:, None]              # prescaled [N, 74]
    tab = np.zeros((N // 2, 2 * P), BF16)
    tab[:, :IN_DIM] = xs[0::2]
    tab[:, P:P + IN_DIM] = xs[1::2]

    core_lo = np.searchsorted(graph_ids, np.arange(0, B + 1, GPC))
    ncore_nodes = core_lo[1:] - core_lo[:-1]
    NT = int(np.ceil(ncore_nodes.max() / P))
    NPAD = NT * P

    # per-core contiguous blocks: self features, rsqrt(deg_in), S matrix
    selfX = np.zeros((NCORES, P, NT, IN_DIM), BF16)
    rdgi = np.ones((NCORES, P, NT), np.float32)
    S = np.zeros((NCORES, P, NT, GPC), BF16)
    for c in range(NCORES):
        lo, hi = int(core_lo[c]), int(core_lo[c + 1])
        n = hi - lo
        v = np.arange(lo, hi)
        t, p = np.arange(n) // P, np.arange(n) % P
        selfX[c, p, t] = xs[v]
        rdgi[c, p, t] = 1.0 / np.sqrt(deg_in[v])
        S[c, p, t, graph_ids[v] - c * GPC] = 1.0

    cnt_g = np.bincount(graph_ids, minlength=B).astype(np.float32)
    assert cnt_g.max() < 256, "graph node count exceeds bf16 exact range"
    ncount = np.ascontiguousarray(cnt_g.reshape(NCORES, 1, GPC).astype(BF16))

    # edge tokens: sorted by (core, dst tile); per-tile block count is the
    # max over cores (SPMD uniform program)
    gid_d = graph_ids[dst]
    ec = gid_d // GPC
    pos = dst - core_lo[ec]
    et, ep = pos // P, pos % P
    cnt = np.zeros((NCORES, NT), np.int64)
    np.add.at(cnt, (ec, et), 1)
    nblk_t = np.ceil(cnt.max(axis=0) / P).astype(np.int64)     # [NT]
    blk0_t = np.concatenate([[0], np.cumsum(nblk_t)])
    NBLK = int(blk0_t[-1])
    NTOK = NBLK * P

    key = ec * NT + et
    order = np.argsort(key, kind="stable")
    ks = key[order]
    starts = np.r_[0, np.flatnonzero(np.diff(ks)) + 1]
    grp_len = np.diff(np.r_[starts, E])
    slot_sorted = np.arange(E) - np.repeat(starts, grp_len)
    slot = np.empty(E, np.int64)
    slot[order] = slot_sorted
    tok = blk0_t[et] * P + slot                       # token index per edge

    idx_flat = np.zeros((NCORES, NTOK), np.int16)
    idx_flat[ec, tok] = (src // 2).astype(np.int16)
    dl = np.full((NCORES, P, 2, NBLK), KILL, BF16)
    dl[ec, tok % P, src % 2, tok // P] = ep.astype(np.float32)

    def wrap(a):  # token-major -> wrapped [128, tokens//16]
        ncol = a.shape[1] // 16
        w = a.reshape(a.shape[0], ncol, 16).transpose(0, 2, 1)
        return np.ascontiguousarray(np.tile(w, (1, 8, 1)))

    idx_wrapped = wrap(idx_flat)

    # one-hot proteins grouped 4/DMA: [PPC//4, 128, LCONV] bf16
    seq = np.asarray(inputs["protein_seq"]).reshape(NCORES, PPC, L)
    oh = np.zeros((NCORES, PPC, 32, LCONV), BF16)
    iot = np.arange(VOCAB)[None, None, :, None]
    oh[:, :, :VOCAB, 1:1 + L] = (seq[:, :, None, :] == iot)
    oh = np.ascontiguousarray(oh.reshape(NCORES, PPC // 4, 4 * 32, LCONV))

    def b16(name):
        return np.asarray(inputs[name], np.float32).astype(BF16)

    shared = {
        "tab": tab,
        "W_gc": b16("W_gc"),
        "b_gc": np.asarray(inputs["b_gc"], np.float32).reshape(HID, 1),
        "W_ro_in": b16("W_ro_in"),
        "b_ro_in": np.asarray(inputs["b_ro_in"], np.float32).reshape(HID, 1),
        "W_ro_out": b16("W_ro_out"),
        "b_ro_row": np.ascontiguousarray(b16("b_ro_out").reshape(1, HID)),
        "Wc1": b16("Wc1"),
        "bc1": np.asarray(inputs["bc1"], np.float32).reshape(HID, 1),
        "Wc2": b16("Wc2"),
        "bc2": np.asarray(inputs["bc2"], np.float32).reshape(HID, 1),
        "embedT": np.ascontiguousarray(b16("embed").T),       # [HID, 25]
        "Wf1_r": np.ascontiguousarray(
            b16("Wf1").reshape(2, HID, 2 * HID)),
        "bf1_r": np.ascontiguousarray(
            np.asarray(inputs["bf1"], np.float32).reshape(2, HID, 1)),
        "Wf2_r": np.ascontiguousarray(b16("Wf2").reshape(2, HID, 1)),
        "bf2": np.asarray(inputs["bf2"], np.float32).reshape(1, 1),
    }
    for l in range(4):
        K = np.asarray(inputs["K%d" % (l + 1)], np.float32)  # [o, i, 3]
        shared["K%dT" % (l + 1)] = np.ascontiguousarray(
            K.transpose(1, 2, 0)).astype(BF16)               # [i, 3, o]
        shared["cb%d" % (l + 1)] = np.asarray(
            inputs["cb%d" % (l + 1)], np.float32).reshape(-1, 1)

    percore = []
    for c in range(NCORES):
        percore.append({
            "selfX": np.ascontiguousarray(selfX[c]),
            "rdgi": np.ascontiguousarray(rdgi[c]),
            "S": np.ascontiguousarray(S[c]),
            "ncount": ncount[c],
            "onehot": np.ascontiguousarray(oh[c]),
            "ix": idx_wrapped[c],
            "dl": np.ascontiguousarray(dl[c]),
        })
    meta = dict(NT=NT, NBLK=NBLK, NTOK=NTOK,
                nblk_t=nblk_t.tolist(), blk0_t=blk0_t.tolist())
    return shared, percore, meta


# --------------------------------------------------------------- device build
def _build(shared, meta):
    NT = meta["NT"]
    NBLK = meta["NBLK"]
    NTOK = meta["NTOK"]
    nblk_t = meta["nblk_t"]
    blk0_t = meta["blk0_t"]
    n_ginstr = (NBLK + BPI - 1) // BPI

    nc = bacc.Bacc("TRN2", target_bir_lowering=False, debug=False,
                   num_devices=NCORES, num_swdge_queues=4)
    f32, bf16, i16 = dt.float32, dt.bfloat16, dt.int16

    D = {k: nc.dram_tensor(k, list(v.shape), dt.from_np(v.dtype),
                           kind="ExternalInput")
         for k, v in shared.items()}
    D["selfX"] = nc.dram_tensor("selfX", [P, NT, IN_DIM], bf16,
                                kind="ExternalInput")
    D["rdgi"] = nc.dram_tensor("rdgi", [P, NT], f32, kind="ExternalInput")
    D["S"] = nc.dram_tensor("S", [P, NT, GPC], bf16, kind="ExternalInput")
    D["ncount"] = nc.dram_tensor("ncount", [1, GPC], bf16,
                                 kind="ExternalInput")
    D["onehot"] = nc.dram_tensor("onehot", [PPC // 4, P, LCONV], bf16,
                                 kind="ExternalInput")
    D["ix"] = nc.dram_tensor("ix", [P, NTOK // 16], i16, kind="ExternalInput")
    D["dl"] = nc.dram_tensor("dl", [P, 2, NBLK], bf16, kind="ExternalInput")
    out_d = nc.dram_tensor("out", [1, GPC], f32, kind="ExternalOutput")
    dbg_pmax = nc.dram_tensor("dbg_pmax", [P, PPC], f32,
                              kind="ExternalOutput") if DEBUG_OUT else None
    dbg_cv = nc.dram_tensor("dbg_cv", [HID, GPC], f32,
                            kind="ExternalOutput") if DEBUG_OUT else None

    with tile.TileContext(nc) as tc, contextlib.ExitStack() as ctx:
        wp = ctx.enter_context(tc.tile_pool(name="wp", bufs=1))
        gp = ctx.enter_context(tc.tile_pool(name="gp", bufs=1))
        selp = ctx.enter_context(tc.tile_pool(name="selp", bufs=1))
        accp = ctx.enter_context(tc.tile_pool(name="accp", bufs=3))
        cvp = ctx.enter_context(tc.tile_pool(name="cvp", bufs=2))
        gnp = ctx.enter_context(tc.tile_pool(name="gnp", bufs=3))
        pcv = ctx.enter_context(tc.tile_pool(name="pcv", bufs=5, space="PSUM"))
        pgn = ctx.enter_context(tc.tile_pool(name="pgn", bufs=2, space="PSUM"))
        phg = ctx.enter_context(tc.tile_pool(name="phg", bufs=1, space="PSUM"))

        # ---------------- setup: weights to SBUF
        def ld(name, shape, dtype=bf16, src=None, tag=None):
            t = wp.tile(shape, dtype, tag=tag or name)
            ap = D[name][:] if src is None else src
            nc.sync.dma_start(out=t[:], in_=ap)
            return t

        # conv-critical + gather-critical loads first so the first protein
        # group and the first gather instr start as early as possible; the
        # agg/GNN/head weights stream in behind them on the sync queue.
        embT = ld("embedT", [HID, VOCAB])
        KT = [ld("K%dT" % (l + 1), [CHANNELS[l], 3, CHANNELS[l + 1]])
              for l in range(4)]
        cb = [ld("cb%d" % (l + 1), [CHANNELS[l + 1], 1], f32)
              for l in range(4)]
        ixt = ld("ix", [P, NTOK // 16], i16)
        dlt = ld("dl", [P, 2, NBLK])

        xb = []
        for l in range(3):
            pair = []
            for j in range(2):
                t = wp.tile([CHANNELS[l + 1], LCONV], bf16,
                            tag="xb%d_%d" % (l, j))
                nc.vector.memset(t[:, 0:1], 0.0)
                nc.vector.memset(t[:, LCONV - 1:LCONV], 0.0)
                pair.append(t)
            xb.append(pair)

        ident = wp.tile([P, P], f32, tag="ident")
        make_identity(nc, ident[:])
        identb = wp.tile([P, P], bf16, tag="identb")
        nc.vector.tensor_copy(identb[:], ident[:])

        iota_big = wp.tile([P, CB, P], bf16, tag="iota_big")
        nc.gpsimd.iota(iota_big[:], [[0, CB], [1, P]], channel_multiplier=0,
                       allow_small_or_imprecise_dtypes=True)

        # M1rep[32s:32s+25, t, :] = embed @ K1_t^T replicated at 4 offsets
        M1rep = wp.tile([P, 3, CHANNELS[1]], bf16, tag="m1rep")
        for t in range(3):
            pm = pgn.tile([VOCAB, CHANNELS[1]], f32, space="PSUM",
                          tag="gps")
            nc.tensor.matmul(pm[:], embT[:], KT[0][:, t, :], start=True,
                             stop=True)
            nc.scalar.copy(M1rep[:VOCAB, t, :], pm[:])
        for srow in range(1, 4):
            nc.sync.dma_start(out=M1rep[32 * srow:32 * srow + VOCAB, :, :],
                              in_=M1rep[:VOCAB, :, :])

        # ---------------- job helpers
        g_tiles = {}          # instr -> sbuf tile
        sel_tiles = {}        # (parity, chunk) -> sbuf tile
        hgst = [False]        # hg_ps accumulation started?
        hg_ps = phg.tile([GPC, HID], f32, space="PSUM", tag="hgps")

        def emit_gather(i):
            b0 = i * BPI
            nb = min(BPI, NBLK - b0)
            ntok = nb * P
            g = gp.tile([P, nb, 2 * P], bf16, tag="g%d" % (i % 3))
            off = b0 * P
            nc.gpsimd.dma_gather(
                out_ap=g[:], in_ap=D["tab"][:],
                idxs_ap=ixt[:, off // 16:(off + ntok) // 16],
                num_idxs=ntok, num_idxs_reg=ntok, elem_size=2 * P,
                single_packet=False, queue_num=i % 4)
            g_tiles[i] = g
            # prebuild the Sel chunks this instr's blocks will need
            for c in range(b0 // CB, (b0 + nb + CB - 1) // CB):
                for par in range(2):
                    if (par, c) in sel_tiles:
                        continue
                    c0 = c * CB
                    cbn = min(CB, NBLK - c0)
                    s = selp.tile([P, cbn, P], bf16,
                                  tag="sel%d_%d" % (par, c % 6))
                    nc.vector.tensor_tensor(
                        out=s[:], in0=iota_big[:, :cbn, :],
                        in1=dlt[:, par, c0:c0 + cbn, None]
                            .to_broadcast([P, cbn, P]),
                        op=ALU.is_equal)
                    sel_tiles[(par, c)] = s

        # start the first gathers now (Pool gen is the long pole for agg
        # readiness), then stream the remaining weight loads behind them
        emit_gather(0)
        if n_ginstr > 1:
            emit_gather(1)

        W_gc = ld("W_gc", [IN_DIM, HID])
        b_gc = ld("b_gc", [HID, 1], f32)
        W_ri = ld("W_ro_in", [HID, HID])
        b_ri = ld("b_ro_in", [HID, 1], f32)
        W_ro = ld("W_ro_out", [HID, HID])
        b_ro_row = ld("b_ro_row", [1, HID])
        Wc1 = ld("Wc1", [HID, HID]); bc1 = ld("bc1", [HID, 1], f32)
        Wc2 = ld("Wc2", [HID, HID]); bc2 = ld("bc2", [HID, 1], f32)
        Wf1 = ld("Wf1_r", [HID, 2, 2 * HID],
                 src=D["Wf1_r"][:].rearrange("k h m -> h k m"))
        bf1 = ld("bf1_r", [HID, 2, 1], f32,
                 src=D["bf1_r"][:].rearrange("k h o -> h k o"))
        Wf2 = ld("Wf2_r", [HID, 2, 1],
                 src=D["Wf2_r"][:].rearrange("k h o -> h k o"))
        bf2 = ld("bf2", [1, 1], f32)
        Sg = ld("S", [P, NT, GPC])
        selfX = ld("selfX", [P, NT, IN_DIM])
        ncnt = ld("ncount", [1, GPC])
        dgin = ld("rdgi", [P, NT], f32)
        rdgi = dgin            # host already sends rsqrt(deg_in)

        def emit_tile_agg(t):
            # aggregate: acc[dst, 74] = sum_blocks Sel^T @ G + I @ selfX
            pa = pgn.tile([P, IN_DIM], f32, space="PSUM", tag="gps")
            first = True
            for b in range(blk0_t[t], blk0_t[t] + nblk_t[t]):
                gi, gl = b // BPI, b % BPI
                ci, cl = b // CB, b % CB
                g = g_tiles[gi]
                for par in range(2):
                    s = sel_tiles[(par, ci)]
                    nc.tensor.matmul(
                        pa[:], s[:, cl, :],
                        g[:, gl, par * P:par * P + IN_DIM],
                        start=first, stop=False)
                    first = False
            nc.tensor.matmul(pa[:], identb[:], selfX[:, t, :],
                             start=first, stop=True)
            acc = accp.tile([P, IN_DIM], bf16, tag="acc")
            nc.vector.tensor_scalar_mul(acc[:], pa[:], rdgi[:, t:t + 1])
            tp = pgn.tile([IN_DIM, P], bf16, space="PSUM", tag="gps")
            nc.tensor.transpose(tp[:], acc[:], identb[:])
            aggT = gnp.tile([IN_DIM, P], bf16, tag="aggT")
            nc.scalar.copy(aggT[:], tp[:])
            return aggT

        def emit_tile_gnn(t, aggT):
            hps = pgn.tile([HID, P], f32, space="PSUM", tag="gps")
            nc.tensor.matmul(hps[:], W_gc[:], aggT[:], start=True, stop=True)
            h = gnp.tile([HID, P], bf16, tag="h")
            nc.scalar.activation(h[:], hps[:], AF.Relu, bias=b_gc[:])
            x1ps = pgn.tile([HID, P], f32, space="PSUM", tag="gps")
            nc.tensor.matmul(x1ps[:], W_ri[:], h[:], start=True, stop=True)
            x1 = gnp.tile([HID, P], bf16, tag="x1")
            nc.vector.tensor_scalar_add(x1[:], x1ps[:], b_ri[:, 0:1])
            x2ps = pgn.tile([P, HID], f32, space="PSUM", tag="gps")
            nc.tensor.matmul(x2ps[:], x1[:], W_ro[:], start=True, stop=True)
            x2n = gnp.tile([P, HID], bf16, tag="x2n")
            nc.scalar.copy(x2n[:], x2ps[:])
            nc.tensor.matmul(hg_ps[:], Sg[:, t, :], x2n[:],
                             start=not hgst[0], stop=False,
                             skip_group_check=True)
            hgst[0] = True

        # ---------------- interleaved: conv proteins + gather/agg jobs
        # A gather instr's SBUF buffer rotates with depth 3 (tag i%3), so
        # every tile whose FIRST block falls in instr i-3 must be emitted
        # before instr i (tiles are in block order; a tile spans <=2 instrs).
        # Each tile splits into an agg job ("ta") and a gnn job ("tg"); the
        # gnn job is delayed one tile so the aggT handoff latency is hidden
        # behind the next tile's agg matmuls.
        jobs = []
        done_tile = [0]

        def tiles_starting_below(blim):
            while done_tile[0] < NT and blk0_t[done_tile[0]] < blim:
                t = done_tile[0]
                jobs.append(("ta", t))
                if t > 0:
                    jobs.append(("tg", t - 1))
                done_tile[0] += 1

        for i in range(2, n_ginstr):
            if i >= 3:
                tiles_starting_below((i - 2) * BPI)
            jobs.append(("g", i))
        tiles_starting_below(NBLK + 1)
        jobs.append(("tg", NT - 1))

        aggT_store = {}

        def run_job(j):
            kind, a = j
            if kind == "g":
                emit_gather(a)
            elif kind == "ta":
                aggT_store[a] = emit_tile_agg(a)
            else:
                emit_tile_gnn(a, aggT_store.pop(a))

        chunkmax = wp.tile([P, 2, PPC], f32, tag="chunkmax")

        def emit_group(grp, after_pair=None):
            # layer-interleaved protein pairs: the PE streams protein p+1's
            # layer while p's activation drains, removing the act-latency
            # stall between layers.
            ohg = cvp.tile([P, LCONV], bf16, tag="ohg")
            nc.sync.dma_start(out=ohg[:], in_=D["onehot"][grp])
            for pair in range(2):
                for l in range(4):
                    cin, cout = CHANNELS[l], CHANNELS[l + 1]
                    for srow in (2 * pair, 2 * pair + 1):
                        p = grp * 4 + srow
                        b0 = 32 * srow
                        xs = xb[l - 1][p % 2] if l > 0 else None
                        for cchunk in range(2):
                            c0 = cchunk * 500
                            pps = pcv.tile([cout, 500], f32, space="PSUM",
                                           tag="cps")
                            for tap in range(3):
                                if l == 0:
                                    lhsT = M1rep[b0:b0 + VOCAB, tap, :]
                                    rhs = ohg[b0:b0 + VOCAB,
                                              c0 + tap:c0 + tap + 500]
                                    tpos = (96, 0) if srow == 3 else None
                                else:
                                    lhsT = KT[l][:, tap, :]
                                    rhs = xs[:cin, c0 + tap:c0 + tap + 500]
                                    tpos = None
                                nc.tensor.matmul(pps[:], lhsT, rhs,
                                                 start=(tap == 0),
                                                 stop=(tap == 2),
                                                 tile_position=tpos)
                            if l == 1:
                                nc.vector.tensor_scalar(
                                    out=xb[l][p % 2][:, 1 + c0:1 + c0 + 500],
                                    in0=pps[:], scalar1=cb[l][:, 0:1],
                                    scalar2=0.0, op0=ALU.add, op1=ALU.max)
                            elif l < 3:
                                nc.scalar.activation(
                                    xb[l][p % 2][:, 1 + c0:1 + c0 + 500],
                                    pps[:], AF.Relu, bias=cb[l][:])
                            else:
                                nc.vector.reduce_max(
                                    out=chunkmax[:, cchunk, p:p + 1],
                                    in_=pps[:, :500], axis=AX.X)
                if after_pair is not None:
                    after_pair(grp * 4 + 2 * pair + 1)

        jq = list(jobs)

        def drain(p):
            while jq and len(jq) > (PPC - 1 - p) * len(jobs) // PPC:
                run_job(jq.pop(0))

        for grp in range(PPC // 4):
            emit_group(grp, after_pair=drain)
        while jq:
            run_job(jq.pop(0))

        # close hg accumulation: += ncount (x) b_ro
        nc.tensor.matmul(hg_ps[:], ncnt[:], b_ro_row[:],
                         start=False, stop=True, skip_group_check=True)

        # pmax = relu(max(chunk maxes) + cb4)
        pmax = wp.tile([P, PPC], bf16, tag="pmax")
        mxt = wp.tile([P, PPC], f32, tag="mxt")
        nc.vector.tensor_reduce(out=mxt[:],
                                in_=chunkmax[:].rearrange("p c q -> p q c"),
                                axis=AX.X, op=ALU.max)
        nc.scalar.activation(pmax[:], mxt[:], AF.Relu, bias=cb[3][:])
        if DEBUG_OUT:
            pmf = wp.tile([P, PPC], f32, tag="pmf")
            nc.vector.tensor_copy(pmf[:], pmax[:])
            nc.sync.dma_start(out=dbg_pmax[:], in_=pmf[:])

        # ---------------- readout + head
        hgT = wp.tile([GPC, HID], bf16, tag="hgT")
        nc.scalar.activation(hgT[:], hg_ps[:], AF.Relu)
        hgt_ps = pgn.tile([HID, GPC], bf16, space="PSUM", tag="gps")
        nc.tensor.transpose(hgt_ps[:], hgT[:], identb[:GPC, :GPC])
        hg = wp.tile([HID, GPC], bf16, tag="hg")
        nc.scalar.copy(hg[:], hgt_ps[:])
        c1ps = pgn.tile([HID, GPC], f32, space="PSUM", tag="gps")
        nc.tensor.matmul(c1ps[:], Wc1[:], hg[:], start=True, stop=True)
        cv1 = wp.tile([HID, GPC], bf16, tag="cv1")
        nc.scalar.activation(cv1[:], c1ps[:], AF.Relu, bias=bc1[:])
        c2ps = pgn.tile([HID, GPC], f32, space="PSUM", tag="gps")
        nc.tensor.matmul(c2ps[:], Wc2[:], cv1[:], start=True, stop=True)
        cv2 = wp.tile([HID, GPC], bf16, tag="cv2")
        nc.scalar.activation(cv2[:], c2ps[:], AF.Relu, bias=bc2[:])
        if DEBUG_OUT:
            cvf = wp.tile([HID, GPC], f32, tag="cvf")
            nc.vector.tensor_copy(cvf[:], cv2[:])
            nc.sync.dma_start(out=dbg_cv[:], in_=cvf[:])
        # head: z = [cv2; pmax]
        zin = [cv2, pmax]
        z2 = []
        for mc in range(2):
            zps = pgn.tile([HID, GPC], f32, space="PSUM", tag="gps")
            for kc in range(2):
                nc.tensor.matmul(zps[:], Wf1[:, kc, mc * HID:(mc + 1) * HID],
                                 zin[kc][:, :GPC], start=(kc == 0),
                                 stop=(kc == 1))
            zt = wp.tile([HID, GPC], bf16, tag="z2_%d" % mc)
            nc.scalar.activation(zt[:], zps[:], AF.Relu, bias=bf1[:, mc, :])
            z2.append(zt)
        ops = pgn.tile([1, GPC], f32, space="PSUM", tag="gps")
        for kc in range(2):
            nc.tensor.matmul(ops[:], Wf2[:, kc, :], z2[kc][:],
                             start=(kc == 0), stop=(kc == 1))
        ot = wp.tile([1, GPC], f32, tag="ot")
        nc.scalar.activation(ot[:], ops[:], AF.Sigmoid, bias=bf2[:1, :])
        nc.sync.dma_start(out=out_d[:], in_=ot[:])

    nc.compile()
    return nc


def kernel(**inputs):
    shared, percore, meta = _host_prep(inputs)
    nc = _build(shared, meta)
    in_maps = []
    for c in range(NCORES):
        m = dict(shared)
        m.update(percore[c])
        in_maps.append(m)
    res = run_bass_kernel_spmd(nc, in_maps, list(range(NCORES)))
    out = np.concatenate([res.results[c]["out"].reshape(GPC)
                          for c in range(NCORES)])
    return out.reshape(B, 1).astype(np.float32)


if __name__ == "__main__":
    sys.path.insert(0, "/root/problem")
    import jax
    import reference
    with jax.default_device(jax.devices("cpu")[0]):
        inputs = {k: np.asarray(v) for k, v in reference.setup_inputs().items()}
        exp = np.asarray(reference.reference(**inputs))
    got = kernel(**inputs)
    err = np.abs(got - exp).max()
    rel = err / max(np.abs(exp).max(), 1e-9)
    print("max abs err:", err, " rel:", rel)


# revision 26
# speedup vs baseline: 3.3892x; 1.0690x over previous
"""CPI_DGLLife kernel for 8 Trainium2 NeuronCores (SPMD).

GCN over a 65536-node graph + protein conv1d branch + CPI head.
Sharding: data-parallel over the 512-graph batch (64 graphs / core).

Aggregation: bf16 pair-row table (2 nodes / 512B row, prescaled by
rsqrt(deg_out)) gathered with exact edge tokens sorted by dst tile;
per-128-token blocks reduced onto dst lanes with one-hot Sel matmuls
(Sel built on-device via is_equal against an iota tile); self loops
added via an identity matmul of a contiguous per-core feature block.
"""
import sys
sys.path.insert(0, "/opt/trn_rl_repo")
import contextlib
import numpy as np

import concourse.bass as bass
import concourse.bacc as bacc
import concourse.tile as tile
from concourse import mybir
from concourse.bass_utils import run_bass_kernel_spmd
from concourse.masks import make_identity

dt = mybir.dt
AF = mybir.ActivationFunctionType
ALU = mybir.AluOpType
AX = mybir.AxisListType
BF16 = mybir.dt.np(dt.bfloat16)

P = 128
N, E, B, L = 65536, 262144, 512, 1000
IN_DIM, HID, VOCAB = 74, 128, 25
CHANNELS = [HID, 96, 128, IN_DIM, HID]
NCORES = 8
GPC = B // NCORES              # graphs per core = 64
PPC = GPC                      # proteins per core = 64
LCONV = 1002                   # 1000 + 2 guard cols
BPI = 32                       # gather blocks per dma_gather instruction
CB = 16                        # blocks per Sel chunk
KILL = 300.0                   # dst-lane code that matches no iota column
DEBUG_OUT = False              # extra pmax/cv2 outputs for error attribution


# ------------------------------------------------------------------ host prep
def _host_prep(inputs):
    graph_ids = np.asarray(inputs["graph_ids"]).astype(np.int64)
    src = np.asarray(inputs["edge_src"]).astype(np.int64)
    dst = np.asarray(inputs["edge_dst"]).astype(np.int64)
    deg_out = np.bincount(src, minlength=N).astype(np.float32) + 1.0
    deg_in = np.bincount(dst, minlength=N).astype(np.float32) + 1.0

    nf = np.asarray(inputs["node_feats"], np.float32)
    xs = nf / np.sqrt(deg_out)[:, None]              # prescaled [N, 74]
    tab = np.zeros((N // 2, 2 * P), BF16)
    tab[:, :IN_DIM] = xs[0::2]
    tab[:, P:P + IN_DIM] = xs[1::2]

    core_lo = np.searchsorted(graph_ids, np.arange(0, B + 1, GPC))
    ncore_nodes = core_lo[1:] - core_lo[:-1]
    NT = int(np.ceil(ncore_nodes.max() / P))
    NPAD = NT * P

    # per-core contiguous blocks: self features, rsqrt(deg_in), S matrix
    selfX = np.zeros((NCORES, P, NT, IN_DIM), BF16)
    rdgi = np.ones((NCORES, P, NT), np.float32)
    S = np.zeros((NCORES, P, NT, GPC), BF16)
    for c in range(NCORES):
        lo, hi = int(core_lo[c]), int(core_lo[c + 1])
        n = hi - lo
        v = np.arange(lo, hi)
        t, p = np.arange(n) // P, np.arange(n) % P
        selfX[c, p, t] = xs[v]
        rdgi[c, p, t] = 1.0 / np.sqrt(deg_in[v])
        S[c, p, t, graph_ids[v] - c * GPC] = 1.0

    cnt_g = np.bincount(graph_ids, minlength=B).astype(np.float32)
    assert cnt_g.max() < 256, "graph node count exceeds bf16 exact range"
    ncount = np.ascontiguousarray(cnt_g.reshape(NCORES, 1, GPC).astype(BF16))

    # edge tokens: sorted by (core, dst tile); per-tile block count is the
    # max over cores (SPMD uniform program)
    gid_d = graph_ids[dst]
    ec = gid_d // GPC
    pos = dst - core_lo[ec]
    et, ep = pos // P, pos % P
    cnt = np.zeros((NCORES, NT), np.int64)
    np.add.at(cnt, (ec, et), 1)
    nblk_t = np.ceil(cnt.max(axis=0) / P).astype(np.int64)     # [NT]
    blk0_t = np.concatenate([[0], np.cumsum(nblk_t)])
    NBLK = int(blk0_t[-1])
    NTOK = NBLK * P

    key = ec * NT + et
    order = np.argsort(key, kind="stable")
    ks = key[order]
    starts = np.r_[0, np.flatnonzero(np.diff(ks)) + 1]
    grp_len = np.diff(np.r_[starts, E])
    slot_sorted = np.arange(E) - np.repeat(starts, grp_len)
    slot = np.empty(E, np.int64)
    slot[order] = slot_sorted
    tok = blk0_t[et] * P + slot                       # token index per edge

    idx_flat = np.zeros((NCORES, NTOK), np.int16)
    idx_flat[ec, tok] = (src // 2).astype(np.int16)
    dl = np.full((NCORES, P, 2, NBLK), KILL, BF16)
    dl[ec, tok % P, src % 2, tok // P] = ep.astype(np.float32)

    def wrap(a):  # token-major -> wrapped [128, tokens//16]
        ncol = a.shape[1] // 16
        w = a.reshape(a.shape[0], ncol, 16).transpose(0, 2, 1)
        return np.ascontiguousarray(np.tile(w, (1, 8, 1)))

    idx_wrapped = wrap(idx_flat)

    # tap-shifted one-hot per protein: oh3[25t+v, j] = (seq[j+t-1] == v),
    # so conv layer 1 is a single 75-row matmul per chunk (taps packed
    # into the contraction dim)
    seq = np.asarray(inputs["protein_seq"]).reshape(NCORES, PPC, L)
    ohb = np.zeros((NCORES, PPC, VOCAB, L + 2), BF16)
    iot = np.arange(VOCAB)[None, None, :, None]
    ohb[:, :, :, 1:1 + L] = (seq[:, :, None, :] == iot)
    oh = np.empty((NCORES, PPC, 3 * VOCAB, L), BF16)
    for t in range(3):
        oh[:, :, VOCAB * t:VOCAB * (t + 1), :] = ohb[:, :, :, t:t + L]
    oh = np.ascontiguousarray(oh)

    def b16(name):
        return np.asarray(inputs[name], np.float32).astype(BF16)

    shared = {
        "tab": tab,
        "W_gc": b16("W_gc"),
        "b_gc": np.asarray(inputs["b_gc"], np.float32).reshape(HID, 1),
        "W_ro_in": b16("W_ro_in"),
        "b_ro_in": np.asarray(inputs["b_ro_in"], np.float32).reshape(HID, 1),
        "W_ro_out": b16("W_ro_out"),
        "b_ro_row": np.ascontiguousarray(b16("b_ro_out").reshape(1, HID)),
        "Wc1": b16("Wc1"),
        "bc1": np.asarray(inputs["bc1"], np.float32).reshape(HID, 1),
        "Wc2": b16("Wc2"),
        "bc2": np.asarray(inputs["bc2"], np.float32).reshape(HID, 1),
        "embedT": np.ascontiguousarray(b16("embed").T),       # [HID, 25]
        "Wf1_r": np.ascontiguousarray(
            b16("Wf1").reshape(2, HID, 2 * HID)),
        "bf1_r": np.ascontiguousarray(
            np.asarray(inputs["bf1"], np.float32).reshape(2, HID, 1)),
        "Wf2_r": np.ascontiguousarray(b16("Wf2").reshape(2, HID, 1)),
        "bf2": np.asarray(inputs["bf2"], np.float32).reshape(1, 1),
    }
    for l in range(4):
        K = np.asarray(inputs["K%d" % (l + 1)], np.float32)  # [o, i, 3]
        shared["K%dT" % (l + 1)] = np.ascontiguousarray(
            K.transpose(1, 2, 0)).astype(BF16)               # [i, 3, o]
        shared["cb%d" % (l + 1)] = np.asarray(
            inputs["cb%d" % (l + 1)], np.float32).reshape(-1, 1)

    percore = []
    for c in range(NCORES):
        percore.append({
            "selfX": np.ascontiguousarray(selfX[c]),
            "rdgi": np.ascontiguousarray(rdgi[c]),
            "S": np.ascontiguousarray(S[c]),
            "ncount": ncount[c],
            "onehot": np.ascontiguousarray(oh[c]),
            "ix": idx_wrapped[c],
            "dl": np.ascontiguousarray(dl[c]),
        })
    meta = dict(NT=NT, NBLK=NBLK, NTOK=NTOK,
                nblk_t=nblk_t.tolist(), blk0_t=blk0_t.tolist())
    return shared, percore, meta


# --------------------------------------------------------------- device build
def _build(shared, meta):
    NT = meta["NT"]
    NBLK = meta["NBLK"]
    NTOK = meta["NTOK"]
    nblk_t = meta["nblk_t"]
    blk0_t = meta["blk0_t"]
    n_ginstr = (NBLK + BPI - 1) // BPI

    nc = bacc.Bacc("TRN2", target_bir_lowering=False, debug=False,
                   num_devices=NCORES, num_swdge_queues=4)
    f32, bf16, i16 = dt.float32, dt.bfloat16, dt.int16

    D = {k: nc.dram_tensor(k, list(v.shape), dt.from_np(v.dtype),
                           kind="ExternalInput")
         for k, v in shared.items()}
    D["selfX"] = nc.dram_tensor("selfX", [P, NT, IN_DIM], bf16,
                                kind="ExternalInput")
    D["rdgi"] = nc.dram_tensor("rdgi", [P, NT], f32, kind="ExternalInput")
    D["S"] = nc.dram_tensor("S", [P, NT, GPC], bf16, kind="ExternalInput")
    D["ncount"] = nc.dram_tensor("ncount", [1, GPC], bf16,
                                 kind="ExternalInput")
    D["onehot"] = nc.dram_tensor("onehot", [PPC, 3 * VOCAB, L], bf16,
                                 kind="ExternalInput")
    D["ix"] = nc.dram_tensor("ix", [P, NTOK // 16], i16, kind="ExternalInput")
    D["dl"] = nc.dram_tensor("dl", [P, 2, NBLK], bf16, kind="ExternalInput")
    out_d = nc.dram_tensor("out", [1, GPC], f32, kind="ExternalOutput")
    dbg_pmax = nc.dram_tensor("dbg_pmax", [P, PPC], f32,
                              kind="ExternalOutput") if DEBUG_OUT else None
    dbg_cv = nc.dram_tensor("dbg_cv", [HID, GPC], f32,
                            kind="ExternalOutput") if DEBUG_OUT else None

    with tile.TileContext(nc) as tc, contextlib.ExitStack() as ctx:
        wp = ctx.enter_context(tc.tile_pool(name="wp", bufs=1))
        gp = ctx.enter_context(tc.tile_pool(name="gp", bufs=1))
        selp = ctx.enter_context(tc.tile_pool(name="selp", bufs=1))
        accp = ctx.enter_context(tc.tile_pool(name="accp", bufs=3))
        cvp = ctx.enter_context(tc.tile_pool(name="cvp", bufs=2))
        gnp = ctx.enter_context(tc.tile_pool(name="gnp", bufs=3))
        pcv = ctx.enter_context(tc.tile_pool(name="pcv", bufs=5, space="PSUM"))
        pgn = ctx.enter_context(tc.tile_pool(name="pgn", bufs=2, space="PSUM"))
        phg = ctx.enter_context(tc.tile_pool(name="phg", bufs=1, space="PSUM"))

        # ---------------- setup: weights to SBUF
        def ld(name, shape, dtype=bf16, src=None, tag=None):
            t = wp.tile(shape, dtype, tag=tag or name)
            ap = D[name][:] if src is None else src
            nc.sync.dma_start(out=t[:], in_=ap)
            return t

        # conv-critical + gather-critical loads first so the first protein
        # group and the first gather instr start as early as possible; the
        # agg/GNN/head weights stream in behind them on the sync queue.
        embT = ld("embedT", [HID, VOCAB])
        KT = [ld("K%dT" % (l + 1), [CHANNELS[l], 3, CHANNELS[l + 1]])
              for l in range(4)]
        cb = [ld("cb%d" % (l + 1), [CHANNELS[l + 1], 1], f32)
              for l in range(4)]
        ixt = ld("ix", [P, NTOK // 16], i16)
        dlt = ld("dl", [P, 2, NBLK])

        xb = []
        for l in range(3):
            pair = []
            for j in range(2):
                t = wp.tile([CHANNELS[l + 1], LCONV], bf16,
                            tag="xb%d_%d" % (l, j))
                nc.vector.memset(t[:, 0:1], 0.0)
                nc.vector.memset(t[:, LCONV - 1:LCONV], 0.0)
                pair.append(t)
            xb.append(pair)

        ident = wp.tile([P, P], f32, tag="ident")
        make_identity(nc, ident[:])
        identb = wp.tile([P, P], bf16, tag="identb")
        nc.vector.tensor_copy(identb[:], ident[:])

        iota_big = wp.tile([P, CB, P], bf16, tag="iota_big")
        nc.gpsimd.iota(iota_big[:], [[0, CB], [1, P]], channel_multiplier=0,
                       allow_small_or_imprecise_dtypes=True)

        # M1all[25t+v, :] = (embed @ K1_t^T)[v, :] — the tap-packed L1 weights
        M1all = wp.tile([3 * VOCAB, CHANNELS[1]], bf16, tag="m1all")
        for t in range(3):
            pm = pgn.tile([VOCAB, CHANNELS[1]], f32, space="PSUM",
                          tag="gps")
            nc.tensor.matmul(pm[:], embT[:], KT[0][:, t, :], start=True,
                             stop=True)
            if t == 0:
                nc.scalar.copy(M1all[:VOCAB, :], pm[:])
            else:
                st = wp.tile([VOCAB, CHANNELS[1]], bf16, tag="m1st%d" % t)
                nc.scalar.copy(st[:], pm[:])
                nc.sync.dma_start(out=M1all[VOCAB * t:VOCAB * (t + 1), :],
                                  in_=st[:])

        # ---------------- job helpers
        g_tiles = {}          # instr -> sbuf tile
        sel_tiles = {}        # (parity, chunk) -> sbuf tile
        hgst = [False]        # hg_ps accumulation started?
        hg_ps = phg.tile([GPC, HID], f32, space="PSUM", tag="hgps")

        def emit_gather(i):
            b0 = i * BPI
            nb = min(BPI, NBLK - b0)
            ntok = nb * P
            g = gp.tile([P, nb, 2 * P], bf16, tag="g%d" % (i % 3))
            off = b0 * P
            nc.gpsimd.dma_gather(
                out_ap=g[:], in_ap=D["tab"][:],
                idxs_ap=ixt[:, off // 16:(off + ntok) // 16],
                num_idxs=ntok, num_idxs_reg=ntok, elem_size=2 * P,
                single_packet=False, queue_num=i % 4)
            g_tiles[i] = g
            # prebuild the Sel chunks this instr's blocks will need
            for c in range(b0 // CB, (b0 + nb + CB - 1) // CB):
                for par in range(2):
                    if (par, c) in sel_tiles:
                        continue
                    c0 = c * CB
                    cbn = min(CB, NBLK - c0)
                    s = selp.tile([P, cbn, P], bf16,
                                  tag="sel%d_%d" % (par, c % 6))
                    nc.vector.tensor_tensor(
                        out=s[:], in0=iota_big[:, :cbn, :],
                        in1=dlt[:, par, c0:c0 + cbn, None]
                            .to_broadcast([P, cbn, P]),
                        op=ALU.is_equal)
                    sel_tiles[(par, c)] = s

        # start the first gathers now (Pool gen is the long pole for agg
        # readiness), then stream the remaining weight loads behind them
        emit_gather(0)
        if n_ginstr > 1:
            emit_gather(1)

        W_gc = ld("W_gc", [IN_DIM, HID])
        b_gc = ld("b_gc", [HID, 1], f32)
        W_ri = ld("W_ro_in", [HID, HID])
        b_ri = ld("b_ro_in", [HID, 1], f32)
        W_ro = ld("W_ro_out", [HID, HID])
        b_ro_row = ld("b_ro_row", [1, HID])
        Wc1 = ld("Wc1", [HID, HID]); bc1 = ld("bc1", [HID, 1], f32)
        Wc2 = ld("Wc2", [HID, HID]); bc2 = ld("bc2", [HID, 1], f32)
        Wf1 = ld("Wf1_r", [HID, 2, 2 * HID],
                 src=D["Wf1_r"][:].rearrange("k h m -> h k m"))
        bf1 = ld("bf1_r", [HID, 2, 1], f32,
                 src=D["bf1_r"][:].rearrange("k h o -> h k o"))
        Wf2 = ld("Wf2_r", [HID, 2, 1],
                 src=D["Wf2_r"][:].rearrange("k h o -> h k o"))
        bf2 = ld("bf2", [1, 1], f32)
        Sg = ld("S", [P, NT, GPC])
        selfX = ld("selfX", [P, NT, IN_DIM])
        ncnt = ld("ncount", [1, GPC])
        dgin = ld("rdgi", [P, NT], f32)
        rdgi = dgin            # host already sends rsqrt(deg_in)

        def emit_tile_agg(t):
            # aggregate: acc[dst, 74] = sum_blocks Sel^T @ G + I @ selfX
            pa = pgn.tile([P, IN_DIM], f32, space="PSUM", tag="gps")
            first = True
            for b in range(blk0_t[t], blk0_t[t] + nblk_t[t]):
                gi, gl = b // BPI, b % BPI
                ci, cl = b // CB, b % CB
                g = g_tiles[gi]
                for par in range(2):
                    s = sel_tiles[(par, ci)]
                    nc.tensor.matmul(
                        pa[:], s[:, cl, :],
                        g[:, gl, par * P:par * P + IN_DIM],
                        start=first, stop=False)
                    first = False
            nc.tensor.matmul(pa[:], identb[:], selfX[:, t, :],
                             start=first, stop=True)
            acc = accp.tile([P, IN_DIM], bf16, tag="acc")
            nc.vector.tensor_scalar_mul(acc[:], pa[:], rdgi[:, t:t + 1])
            tp = pgn.tile([IN_DIM, P], bf16, space="PSUM", tag="gps")
            nc.tensor.transpose(tp[:], acc[:], identb[:])
            aggT = gnp.tile([IN_DIM, P], bf16, tag="aggT")
            nc.scalar.copy(aggT[:], tp[:])
            return aggT

        def emit_tile_gnn(t, aggT):
            hps = pgn.tile([HID, P], f32, space="PSUM", tag="gps")
            nc.tensor.matmul(hps[:], W_gc[:], aggT[:], start=True, stop=True)
            h = gnp.tile([HID, P], bf16, tag="h")
            nc.scalar.activation(h[:], hps[:], AF.Relu, bias=b_gc[:])
            x1ps = pgn.tile([HID, P], f32, space="PSUM", tag="gps")
            nc.tensor.matmul(x1ps[:], W_ri[:], h[:], start=True, stop=True)
            x1 = gnp.tile([HID, P], bf16, tag="x1")
            nc.vector.tensor_scalar_add(x1[:], x1ps[:], b_ri[:, 0:1])
            x2ps = pgn.tile([P, HID], f32, space="PSUM", tag="gps")
            nc.tensor.matmul(x2ps[:], x1[:], W_ro[:], start=True, stop=True)
            x2n = gnp.tile([P, HID], bf16, tag="x2n")
            nc.scalar.copy(x2n[:], x2ps[:])
            nc.tensor.matmul(hg_ps[:], Sg[:, t, :], x2n[:],
                             start=not hgst[0], stop=False,
                             skip_group_check=True)
            hgst[0] = True

        # ---------------- interleaved: conv proteins + gather/agg jobs
        # A gather instr's SBUF buffer rotates with depth 3 (tag i%3), so
        # every tile whose FIRST block falls in instr i-3 must be emitted
        # before instr i (tiles are in block order; a tile spans <=2 instrs).
        # Each tile splits into an agg job ("ta") and a gnn job ("tg"); the
        # gnn job is delayed one tile so the aggT handoff latency is hidden
        # behind the next tile's agg matmuls.
        jobs = []
        done_tile = [0]

        def tiles_starting_below(blim):
            while done_tile[0] < NT and blk0_t[done_tile[0]] < blim:
                t = done_tile[0]
                jobs.append(("ta", t))
                if t > 0:
                    jobs.append(("tg", t - 1))
                done_tile[0] += 1

        for i in range(2, n_ginstr):
            if i >= 3:
                tiles_starting_below((i - 2) * BPI)
            jobs.append(("g", i))
        tiles_starting_below(NBLK + 1)
        jobs.append(("tg", NT - 1))

        aggT_store = {}

        def run_job(j):
            kind, a = j
            if kind == "g":
                emit_gather(a)
            elif kind == "ta":
                aggT_store[a] = emit_tile_agg(a)
            else:
                emit_tile_gnn(a, aggT_store.pop(a))

        chunkmax = wp.tile([P, 2, PPC], f32, tag="chunkmax")

        def emit_group(grp, after_pair=None):
            # layer-interleaved protein pairs: the PE streams protein p+1's
            # layer while p's activation drains, removing the act-latency
            # stall between layers.
            ohts = {}
            for sp in range(4):
                p = grp * 4 + sp
                oht = cvp.tile([3 * VOCAB, L], bf16, tag="ohg%d" % (p % 4))
                nc.sync.dma_start(out=oht[:], in_=D["onehot"][p])
                ohts[sp] = oht
            for pair in range(2):
                for l in range(4):
                    cin, cout = CHANNELS[l], CHANNELS[l + 1]
                    for srow in (2 * pair, 2 * pair + 1):
                        p = grp * 4 + srow
                        xs = xb[l - 1][p % 2] if l > 0 else None
                        for cchunk in range(2):
                            c0 = cchunk * 500
                            pps = pcv.tile([cout, 500], f32, space="PSUM",
                                           tag="cps")
                            if l == 0:
                                nc.tensor.matmul(pps[:], M1all[:],
                                                 ohts[srow][:, c0:c0 + 500],
                                                 start=True, stop=True)
                            else:
                                for tap in range(3):
                                    nc.tensor.matmul(
                                        pps[:], KT[l][:, tap, :],
                                        xs[:cin, c0 + tap:c0 + tap + 500],
                                        start=(tap == 0), stop=(tap == 2))
                            if l < 3:
                                nc.scalar.activation(
                                    xb[l][p % 2][:, 1 + c0:1 + c0 + 500],
                                    pps[:], AF.Relu, bias=cb[l][:])
                            else:
                                nc.vector.reduce_max(
                                    out=chunkmax[:, cchunk, p:p + 1],
                                    in_=pps[:, :500], axis=AX.X)
                if after_pair is not None:
                    after_pair(grp * 4 + 2 * pair + 1)

        jq = list(jobs)

        def drain(p):
            while jq and len(jq) > (PPC - 1 - p) * len(jobs) // PPC:
                run_job(jq.pop(0))

        for grp in range(PPC // 4):
            emit_group(grp, after_pair=drain)
        while jq:
            run_job(jq.pop(0))

        # close hg accumulation: += ncount (x) b_ro
        nc.tensor.matmul(hg_ps[:], ncnt[:], b_ro_row[:],
                         start=False, stop=True, skip_group_check=True)

        # pmax = relu(max(chunk maxes) + cb4)
        pmax = wp.tile([P, PPC], bf16, tag="pmax")
        mxt = wp.tile([P, PPC], f32, tag="mxt")
        nc.vector.tensor_reduce(out=mxt[:],
                                in_=chunkmax[:].rearrange("p c q -> p q c"),
                                axis=AX.X, op=ALU.max)
        nc.scalar.activation(pmax[:], mxt[:], AF.Relu, bias=cb[3][:])
        if DEBUG_OUT:
            pmf = wp.tile([P, PPC], f32, tag="pmf")
            nc.vector.tensor_copy(pmf[:], pmax[:])
            nc.sync.dma_start(out=dbg_pmax[:], in_=pmf[:])

        # ---------------- readout + head
        hgT = wp.tile([GPC, HID], bf16, tag="hgT")
        nc.scalar.activation(hgT[:], hg_ps[:], AF.Relu)
        hgt_ps = pgn.tile([HID, GPC], bf16, space="PSUM", tag="gps")
        nc.tensor.transpose(hgt_ps[:], hgT[:], identb[:GPC, :GPC])
        hg = wp.tile([HID, GPC], bf16, tag="hg")
        nc.scalar.copy(hg[:], hgt_ps[:])
        c1ps = pgn.tile([HID, GPC], f32, space="PSUM", tag="gps")
        nc.tensor.matmul(c1ps[:], Wc1[:], hg[:], start=True, stop=True)
        cv1 = wp.tile([HID, GPC], bf16, tag="cv1")
        nc.scalar.activation(cv1[:], c1ps[:], AF.Relu, bias=bc1[:])
        c2ps = pgn.tile([HID, GPC], f32, space="PSUM", tag="gps")
        nc.tensor.matmul(c2ps[:], Wc2[:], cv1[:], start=True, stop=True)
        cv2 = wp.tile([HID, GPC], bf16, tag="cv2")
        nc.scalar.activation(cv2[:], c2ps[:], AF.Relu, bias=bc2[:])
        if DEBUG_OUT:
            cvf = wp.tile([HID, GPC], f32, tag="cvf")
            nc.vector.tensor_copy(cvf[:], cv2[:])
            nc.sync.dma_start(out=dbg_cv[:], in_=cvf[:])
        # head: z = [cv2; pmax]
        zin = [cv2, pmax]
        z2 = []
        for mc in range(2):
            zps = pgn.tile([HID, GPC], f32, space="PSUM", tag="gps")
            for kc in range(2):
                nc.tensor.matmul(zps[:], Wf1[:, kc, mc * HID:(mc + 1) * HID],
                                 zin[kc][:, :GPC], start=(kc == 0),
                                 stop=(kc == 1))
            zt = wp.tile([HID, GPC], bf16, tag="z2_%d" % mc)
            nc.scalar.activation(zt[:], zps[:], AF.Relu, bias=bf1[:, mc, :])
            z2.append(zt)
        ops = pgn.tile([1, GPC], f32, space="PSUM", tag="gps")
        for kc in range(2):
            nc.tensor.matmul(ops[:], Wf2[:, kc, :], z2[kc][:],
                             start=(kc == 0), stop=(kc == 1))
        ot = wp.tile([1, GPC], f32, tag="ot")
        nc.scalar.activation(ot[:], ops[:], AF.Sigmoid, bias=bf2[:1, :])
        nc.sync.dma_start(out=out_d[:], in_=ot[:])

    nc.compile()
    return nc


def kernel(**inputs):
    shared, percore, meta = _host_prep(inputs)
    nc = _build(shared, meta)
    in_maps = []
    for c in range(NCORES):
        m = dict(shared)
        m.update(percore[c])
        in_maps.append(m)
    res = run_bass_kernel_spmd(nc, in_maps, list(range(NCORES)))
    out = np.concatenate([res.results[c]["out"].reshape(GPC)
                          for c in range(NCORES)])
    return out.reshape(B, 1).astype(np.float32)


if __name__ == "__main__":
    sys.path.insert(0, "/root/problem")
    import jax
    import reference
    with jax.default_device(jax.devices("cpu")[0]):
        inputs = {k: np.asarray(v) for k, v in reference.setup_inputs().items()}
        exp = np.asarray(reference.reference(**inputs))
    got = kernel(**inputs)
    err = np.abs(got - exp).max()
    rel = err / max(np.abs(exp).max(), 1e-9)
    print("max abs err:", err, " rel:", rel)


# revision 44
# speedup vs baseline: 3.6573x; 1.0791x over previous
"""CPI_DGLLife kernel for 8 Trainium2 NeuronCores (SPMD).

GCN over a 65536-node graph + protein conv1d branch + CPI head.
Sharding: data-parallel over the 512-graph batch (64 graphs / core).

Aggregation: bf16 pair-row table (2 nodes / 512B row, prescaled by
rsqrt(deg_out)) gathered with exact edge tokens sorted by dst tile;
per-128-token blocks reduced onto dst lanes with one-hot Sel matmuls
(Sel built on-device via is_equal against an iota tile); self loops
added via an identity matmul of a contiguous per-core feature block.
"""
import sys
sys.path.insert(0, "/opt/trn_rl_repo")
import contextlib
import numpy as np

import concourse.bass as bass
import concourse.bacc as bacc
import concourse.tile as tile
from concourse import mybir
from concourse.bass_utils import run_bass_kernel_spmd
from concourse.masks import make_identity

dt = mybir.dt
AF = mybir.ActivationFunctionType
ALU = mybir.AluOpType
AX = mybir.AxisListType
BF16 = mybir.dt.np(dt.bfloat16)

P = 128
N, E, B, L = 65536, 262144, 512, 1000
IN_DIM, HID, VOCAB = 74, 128, 25
CHANNELS = [HID, 96, 128, IN_DIM, HID]
NCORES = 8
GPC = B // NCORES              # graphs per core = 64
PPC = GPC                      # proteins per core = 64
LCONV = 1002                   # 1000 + 2 guard cols
BPI = 32                       # gather blocks per dma_gather instruction
CB = 16                        # blocks per Sel chunk
KILL = 300.0                   # dst-lane code that matches no iota column
DEBUG_OUT = False              # extra pmax/cv2 outputs for error attribution


# ------------------------------------------------------------------ host prep
def _host_prep(inputs):
    graph_ids = np.asarray(inputs["graph_ids"]).astype(np.int64)
    src = np.asarray(inputs["edge_src"]).astype(np.int64)
    dst = np.asarray(inputs["edge_dst"]).astype(np.int64)
    deg_out = np.bincount(src, minlength=N).astype(np.float32) + 1.0
    deg_in = np.bincount(dst, minlength=N).astype(np.float32) + 1.0

    nf = np.asarray(inputs["node_feats"], np.float32)
    xs = nf / np.sqrt(deg_out)[:, None]              # prescaled [N, 74]
    tab = np.zeros((N // 2, 2 * P), BF16)
    tab[:, :IN_DIM] = xs[0::2]
    tab[:, P:P + IN_DIM] = xs[1::2]

    core_lo = np.searchsorted(graph_ids, np.arange(0, B + 1, GPC))
    ncore_nodes = core_lo[1:] - core_lo[:-1]
    NT = int(np.ceil(ncore_nodes.max() / P))
    NPAD = NT * P

    # per-core contiguous blocks: self features, rsqrt(deg_in), S matrix
    selfX = np.zeros((NCORES, P, NT, IN_DIM), BF16)
    rdgi = np.ones((NCORES, P, NT), np.float32)
    S = np.zeros((NCORES, P, NT, GPC), BF16)
    for c in range(NCORES):
        lo, hi = int(core_lo[c]), int(core_lo[c + 1])
        n = hi - lo
        v = np.arange(lo, hi)
        t, p = np.arange(n) // P, np.arange(n) % P
        selfX[c, p, t] = xs[v]
        rdgi[c, p, t] = 1.0 / np.sqrt(deg_in[v])
        S[c, p, t, graph_ids[v] - c * GPC] = 1.0

    cnt_g = np.bincount(graph_ids, minlength=B).astype(np.float32)
    assert cnt_g.max() < 256, "graph node count exceeds bf16 exact range"
    ncount = np.ascontiguousarray(cnt_g.reshape(NCORES, 1, GPC).astype(BF16))

    # edge tokens: sorted by (core, dst tile); per-tile block count is the
    # max over cores (SPMD uniform program)
    gid_d = graph_ids[dst]
    ec = gid_d // GPC
    pos = dst - core_lo[ec]
    et, ep = pos // P, pos % P
    cnt = np.zeros((NCORES, NT), np.int64)
    np.add.at(cnt, (ec, et), 1)
    nblk_t = np.ceil(cnt.max(axis=0) / P).astype(np.int64)     # [NT]
    blk0_t = np.concatenate([[0], np.cumsum(nblk_t)])
    NBLK = int(blk0_t[-1])
    NTOK = NBLK * P

    key = ec * NT + et
    order = np.argsort(key, kind="stable")
    ks = key[order]
    starts = np.r_[0, np.flatnonzero(np.diff(ks)) + 1]
    grp_len = np.diff(np.r_[starts, E])
    slot_sorted = np.arange(E) - np.repeat(starts, grp_len)
    slot = np.empty(E, np.int64)
    slot[order] = slot_sorted
    tok = blk0_t[et] * P + slot                       # token index per edge

    idx_flat = np.zeros((NCORES, NTOK), np.int16)
    idx_flat[ec, tok] = (src // 2).astype(np.int16)
    dl = np.full((NCORES, P, 2, NBLK), KILL, BF16)
    dl[ec, tok % P, src % 2, tok // P] = ep.astype(np.float32)

    def wrap(a):  # token-major -> wrapped [128, tokens//16]
        ncol = a.shape[1] // 16
        w = a.reshape(a.shape[0], ncol, 16).transpose(0, 2, 1)
        return np.ascontiguousarray(np.tile(w, (1, 8, 1)))

    idx_wrapped = wrap(idx_flat)

    # tap-shifted one-hot per protein: oh3[25t+v, j] = (seq[j+t-1] == v),
    # so conv layer 1 is a single 75-row matmul per chunk (taps packed
    # into the contraction dim)
    seq = np.asarray(inputs["protein_seq"]).reshape(NCORES, PPC, L)
    ohb = np.zeros((NCORES, PPC, VOCAB, L + 2), BF16)
    iot = np.arange(VOCAB)[None, None, :, None]
    ohb[:, :, :, 1:1 + L] = (seq[:, :, None, :] == iot)
    oh = np.empty((NCORES, PPC, 3 * VOCAB, L), BF16)
    for t in range(3):
        oh[:, :, VOCAB * t:VOCAB * (t + 1), :] = ohb[:, :, :, t:t + L]
    oh = np.ascontiguousarray(oh)

    def b16(name):
        return np.asarray(inputs[name], np.float32).astype(BF16)

    shared = {
        "tab": tab,
        "W_gc": b16("W_gc"),
        "b_gc": np.asarray(inputs["b_gc"], np.float32).reshape(HID, 1),
        "W_ro_in": b16("W_ro_in"),
        "b_ro_in": np.asarray(inputs["b_ro_in"], np.float32).reshape(HID, 1),
        "W_ro_out": b16("W_ro_out"),
        "b_ro_row": np.ascontiguousarray(b16("b_ro_out").reshape(1, HID)),
        "Wc1": b16("Wc1"),
        "bc1": np.asarray(inputs["bc1"], np.float32).reshape(HID, 1),
        "Wc2": b16("Wc2"),
        "bc2": np.asarray(inputs["bc2"], np.float32).reshape(HID, 1),
        "embedT": np.ascontiguousarray(b16("embed").T),       # [HID, 25]
        "Wf1_r": np.ascontiguousarray(
            b16("Wf1").reshape(2, HID, 2 * HID)),
        "bf1_r": np.ascontiguousarray(
            np.asarray(inputs["bf1"], np.float32).reshape(2, HID, 1)),
        "Wf2_r": np.ascontiguousarray(b16("Wf2").reshape(2, HID, 1)),
        "bf2": np.asarray(inputs["bf2"], np.float32).reshape(1, 1),
    }
    for l in range(4):
        K = np.asarray(inputs["K%d" % (l + 1)], np.float32)  # [o, i, 3]
        shared["K%dT" % (l + 1)] = np.ascontiguousarray(
            K.transpose(1, 2, 0)).astype(BF16)               # [i, 3, o]
        shared["cb%d" % (l + 1)] = np.asarray(
            inputs["cb%d" % (l + 1)], np.float32).reshape(-1, 1)

    percore = []
    for c in range(NCORES):
        percore.append({
            "selfX": np.ascontiguousarray(selfX[c]),
            "rdgi": np.ascontiguousarray(rdgi[c]),
            "S": np.ascontiguousarray(S[c]),
            "ncount": ncount[c],
            "onehot": np.ascontiguousarray(oh[c]),
            "ix": idx_wrapped[c],
            "dl": np.ascontiguousarray(dl[c]),
        })
    meta = dict(NT=NT, NBLK=NBLK, NTOK=NTOK,
                nblk_t=nblk_t.tolist(), blk0_t=blk0_t.tolist())
    return shared, percore, meta


# --------------------------------------------------------------- device build
def _build(shared, meta):
    NT = meta["NT"]
    NBLK = meta["NBLK"]
    NTOK = meta["NTOK"]
    nblk_t = meta["nblk_t"]
    blk0_t = meta["blk0_t"]
    n_ginstr = (NBLK + BPI - 1) // BPI

    nc = bacc.Bacc("TRN2", target_bir_lowering=False, debug=False,
                   num_devices=NCORES, num_swdge_queues=4)
    f32, bf16, i16 = dt.float32, dt.bfloat16, dt.int16

    D = {k: nc.dram_tensor(k, list(v.shape), dt.from_np(v.dtype),
                           kind="ExternalInput")
         for k, v in shared.items()}
    D["selfX"] = nc.dram_tensor("selfX", [P, NT, IN_DIM], bf16,
                                kind="ExternalInput")
    D["rdgi"] = nc.dram_tensor("rdgi", [P, NT], f32, kind="ExternalInput")
    D["S"] = nc.dram_tensor("S", [P, NT, GPC], bf16, kind="ExternalInput")
    D["ncount"] = nc.dram_tensor("ncount", [1, GPC], bf16,
                                 kind="ExternalInput")
    D["onehot"] = nc.dram_tensor("onehot", [PPC, 3 * VOCAB, L], bf16,
                                 kind="ExternalInput")
    D["ix"] = nc.dram_tensor("ix", [P, NTOK // 16], i16, kind="ExternalInput")
    D["dl"] = nc.dram_tensor("dl", [P, 2, NBLK], bf16, kind="ExternalInput")
    out_d = nc.dram_tensor("out", [1, GPC], f32, kind="ExternalOutput")
    dbg_pmax = nc.dram_tensor("dbg_pmax", [P, PPC], f32,
                              kind="ExternalOutput") if DEBUG_OUT else None
    dbg_cv = nc.dram_tensor("dbg_cv", [HID, GPC], f32,
                            kind="ExternalOutput") if DEBUG_OUT else None

    with tile.TileContext(nc) as tc, contextlib.ExitStack() as ctx:
        wp = ctx.enter_context(tc.tile_pool(name="wp", bufs=1))
        gp = ctx.enter_context(tc.tile_pool(name="gp", bufs=1))
        selp = ctx.enter_context(tc.tile_pool(name="selp", bufs=1))
        accp = ctx.enter_context(tc.tile_pool(name="accp", bufs=3))
        cvp = ctx.enter_context(tc.tile_pool(name="cvp", bufs=2))
        gnp = ctx.enter_context(tc.tile_pool(name="gnp", bufs=3))
        pcv = ctx.enter_context(tc.tile_pool(name="pcv", bufs=5, space="PSUM"))
        pgn = ctx.enter_context(tc.tile_pool(name="pgn", bufs=2, space="PSUM"))
        phg = ctx.enter_context(tc.tile_pool(name="phg", bufs=1, space="PSUM"))

        # ---------------- setup: weights to SBUF
        def ld(name, shape, dtype=bf16, src=None, tag=None):
            t = wp.tile(shape, dtype, tag=tag or name)
            ap = D[name][:] if src is None else src
            nc.sync.dma_start(out=t[:], in_=ap)
            return t

        # conv-critical + gather-critical loads first so the first protein
        # group and the first gather instr start as early as possible; the
        # agg/GNN/head weights stream in behind them on the sync queue.
        embT = ld("embedT", [HID, VOCAB])
        KT = [ld("K%dT" % (l + 1), [CHANNELS[l], 3, CHANNELS[l + 1]])
              for l in range(4)]
        cb = [ld("cb%d" % (l + 1), [CHANNELS[l + 1], 1], f32)
              for l in range(4)]
        ixt = ld("ix", [P, NTOK // 16], i16)
        dlt = ld("dl", [P, 2, NBLK])

        xb = []
        for l in range(3):
            pair = []
            for j in range(2):
                t = wp.tile([CHANNELS[l + 1], LCONV], bf16,
                            tag="xb%d_%d" % (l, j))
                nc.vector.memset(t[:, 0:1], 0.0)
                nc.vector.memset(t[:, LCONV - 1:LCONV], 0.0)
                pair.append(t)
            xb.append(pair)

        ident = wp.tile([P, P], f32, tag="ident")
        make_identity(nc, ident[:])
        identb = wp.tile([P, P], bf16, tag="identb")
        nc.vector.tensor_copy(identb[:], ident[:])

        iota_big = wp.tile([P, CB, P], bf16, tag="iota_big")
        nc.gpsimd.iota(iota_big[:], [[0, CB], [1, P]], channel_multiplier=0,
                       allow_small_or_imprecise_dtypes=True)

        # M1all[25t+v, :] = (embed @ K1_t^T)[v, :] — the tap-packed L1 weights
        M1all = wp.tile([3 * VOCAB, CHANNELS[1]], bf16, tag="m1all")
        for t in range(3):
            pm = pgn.tile([VOCAB, CHANNELS[1]], f32, space="PSUM",
                          tag="gps")
            nc.tensor.matmul(pm[:], embT[:], KT[0][:, t, :], start=True,
                             stop=True)
            if t == 0:
                nc.scalar.copy(M1all[:VOCAB, :], pm[:])
            else:
                st = wp.tile([VOCAB, CHANNELS[1]], bf16, tag="m1st%d" % t)
                nc.scalar.copy(st[:], pm[:])
                nc.sync.dma_start(out=M1all[VOCAB * t:VOCAB * (t + 1), :],
                                  in_=st[:])

        # ---------------- job helpers
        g_tiles = {}          # instr -> sbuf tile
        sel_tiles = {}        # (parity, chunk) -> sbuf tile
        hgst = [False]        # hg_ps accumulation started?
        hg_ps = phg.tile([GPC, HID], f32, space="PSUM", tag="hgps")

        def emit_gather(i):
            b0 = i * BPI
            nb = min(BPI, NBLK - b0)
            ntok = nb * P
            g = gp.tile([P, nb, 2 * P], bf16, tag="g%d" % (i % 3))
            off = b0 * P
            nc.gpsimd.dma_gather(
                out_ap=g[:], in_ap=D["tab"][:],
                idxs_ap=ixt[:, off // 16:(off + ntok) // 16],
                num_idxs=ntok, num_idxs_reg=ntok, elem_size=2 * P,
                single_packet=False, queue_num=i % 4)
            g_tiles[i] = g
            # prebuild the Sel chunks this instr's blocks will need
            for c in range(b0 // CB, (b0 + nb + CB - 1) // CB):
                for par in range(2):
                    if (par, c) in sel_tiles:
                        continue
                    c0 = c * CB
                    cbn = min(CB, NBLK - c0)
                    s = selp.tile([P, cbn, P], bf16,
                                  tag="sel%d_%d" % (par, c % 6))
                    nc.vector.tensor_tensor(
                        out=s[:], in0=iota_big[:, :cbn, :],
                        in1=dlt[:, par, c0:c0 + cbn, None]
                            .to_broadcast([P, cbn, P]),
                        op=ALU.is_equal)
                    sel_tiles[(par, c)] = s

        # start the first gathers now (Pool gen is the long pole for agg
        # readiness), then stream the remaining weight loads behind them
        emit_gather(0)
        if n_ginstr > 1:
            emit_gather(1)

        W_gc = ld("W_gc", [IN_DIM, HID])
        b_gc = ld("b_gc", [HID, 1], f32)
        W_ri = ld("W_ro_in", [HID, HID])
        b_ri = ld("b_ro_in", [HID, 1], f32)
        W_ro = ld("W_ro_out", [HID, HID])
        b_ro_row = ld("b_ro_row", [1, HID])
        Wc1 = ld("Wc1", [HID, HID]); bc1 = ld("bc1", [HID, 1], f32)
        Wc2 = ld("Wc2", [HID, HID]); bc2 = ld("bc2", [HID, 1], f32)
        Wf1 = ld("Wf1_r", [HID, 2, 2 * HID],
                 src=D["Wf1_r"][:].rearrange("k h m -> h k m"))
        bf1 = ld("bf1_r", [HID, 2, 1], f32,
                 src=D["bf1_r"][:].rearrange("k h o -> h k o"))
        Wf2 = ld("Wf2_r", [HID, 2, 1],
                 src=D["Wf2_r"][:].rearrange("k h o -> h k o"))
        bf2 = ld("bf2", [1, 1], f32)
        Sg = ld("S", [P, NT, GPC])
        selfX = ld("selfX", [P, NT, IN_DIM])
        ncnt = ld("ncount", [1, GPC])
        dgin = ld("rdgi", [P, NT], f32)
        rdgi = dgin            # host already sends rsqrt(deg_in)

        def emit_tile_agg(t):
            # aggregate: acc[dst, 74] = sum_blocks Sel^T @ G + I @ selfX
            pa = pgn.tile([P, IN_DIM], f32, space="PSUM", tag="gps")
            first = True
            for b in range(blk0_t[t], blk0_t[t] + nblk_t[t]):
                gi, gl = b // BPI, b % BPI
                ci, cl = b // CB, b % CB
                g = g_tiles[gi]
                for par in range(2):
                    s = sel_tiles[(par, ci)]
                    nc.tensor.matmul(
                        pa[:], s[:, cl, :],
                        g[:, gl, par * P:par * P + IN_DIM],
                        start=first, stop=False)
                    first = False
            nc.tensor.matmul(pa[:], identb[:], selfX[:, t, :],
                             start=first, stop=True)
            acc = accp.tile([P, IN_DIM], bf16, tag="acc")
            nc.vector.tensor_scalar_mul(acc[:], pa[:], rdgi[:, t:t + 1])
            tp = pgn.tile([IN_DIM, P], bf16, space="PSUM", tag="gps")
            nc.tensor.transpose(tp[:], acc[:], identb[:])
            aggT = gnp.tile([IN_DIM, P], bf16, tag="aggT")
            nc.scalar.copy(aggT[:], tp[:])
            return aggT

        def emit_tile_gnn(t, aggT):
            hps = pgn.tile([HID, P], f32, space="PSUM", tag="gps")
            nc.tensor.matmul(hps[:], W_gc[:], aggT[:], start=True, stop=True)
            h = gnp.tile([HID, P], bf16, tag="h")
            nc.scalar.activation(h[:], hps[:], AF.Relu, bias=b_gc[:])
            x1ps = pgn.tile([HID, P], f32, space="PSUM", tag="gps")
            nc.tensor.matmul(x1ps[:], W_ri[:], h[:], start=True, stop=True)
            x1 = gnp.tile([HID, P], bf16, tag="x1")
            nc.vector.tensor_scalar_add(x1[:], x1ps[:], b_ri[:, 0:1])
            x2ps = pgn.tile([P, HID], f32, space="PSUM", tag="gps")
            nc.tensor.matmul(x2ps[:], x1[:], W_ro[:], start=True, stop=True)
            x2n = gnp.tile([P, HID], bf16, tag="x2n")
            nc.scalar.copy(x2n[:], x2ps[:])
            nc.tensor.matmul(hg_ps[:], Sg[:, t, :], x2n[:],
                             start=not hgst[0], stop=False,
                             skip_group_check=True)
            hgst[0] = True

        # ---------------- interleaved: conv proteins + gather/agg jobs
        # A gather instr's SBUF buffer rotates with depth 3 (tag i%3), so
        # every tile whose FIRST block falls in instr i-3 must be emitted
        # before instr i (tiles are in block order; a tile spans <=2 instrs).
        # Each tile splits into an agg job ("ta") and a gnn job ("tg"); the
        # gnn job is delayed one tile so the aggT handoff latency is hidden
        # behind the next tile's agg matmuls.
        jobs = []
        done_tile = [0]

        def tiles_starting_below(blim):
            while done_tile[0] < NT and blk0_t[done_tile[0]] < blim:
                t = done_tile[0]
                jobs.append(("ta", t))
                if t > 0:
                    jobs.append(("tg", t - 1))
                done_tile[0] += 1

        for i in range(2, n_ginstr):
            if i >= 3:
                tiles_starting_below((i - 2) * BPI)
            jobs.append(("g", i))
        tiles_starting_below(NBLK + 1)
        jobs.append(("tg", NT - 1))

        aggT_store = {}

        def run_job(j):
            kind, a = j
            if kind == "g":
                emit_gather(a)
            elif kind == "ta":
                aggT_store[a] = emit_tile_agg(a)
            else:
                emit_tile_gnn(a, aggT_store.pop(a))

        chunkmax = wp.tile([P, 2, PPC], f32, tag="chunkmax")

        def emit_group(grp, after_pair=None):
            # layer-interleaved protein pairs: the PE streams protein p+1's
            # layer while p's activation drains, removing the act-latency
            # stall between layers.
            ohts = {}
            for sp in range(4):
                p = grp * 4 + sp
                oht = cvp.tile([3 * VOCAB, L], bf16, tag="ohg%d" % (p % 4))
                nc.sync.dma_start(out=oht[:], in_=D["onehot"][p])
                ohts[sp] = oht
            for pair in range(2):
                for l in range(4):
                    cin, cout = CHANNELS[l], CHANNELS[l + 1]
                    for srow in (2 * pair, 2 * pair + 1):
                        p = grp * 4 + srow
                        xs = xb[l - 1][p % 2] if l > 0 else None
                        for cchunk in range(2):
                            c0 = cchunk * 500
                            pps = pcv.tile([cout, 500], f32, space="PSUM",
                                           tag="cps")
                            if l == 0:
                                nc.tensor.matmul(pps[:], M1all[:],
                                                 ohts[srow][:, c0:c0 + 500],
                                                 start=True, stop=True)
                            else:
                                for tap in range(3):
                                    nc.tensor.matmul(
                                        pps[:], KT[l][:, tap, :],
                                        xs[:cin, c0 + tap:c0 + tap + 500],
                                        start=(tap == 0), stop=(tap == 2))
                            if l < 3:
                                nc.scalar.activation(
                                    xb[l][p % 2][:, 1 + c0:1 + c0 + 500],
                                    pps[:], AF.Relu, bias=cb[l][:])
                            else:
                                nc.vector.reduce_max(
                                    out=chunkmax[:, cchunk, p:p + 1],
                                    in_=pps[:, :500], axis=AX.X)
                if after_pair is not None:
                    after_pair(grp * 4 + 2 * pair + 1)

        jq = list(jobs)

        def drain(p):
            while jq and len(jq) > (PPC - 1 - p) * len(jobs) // PPC:
                run_job(jq.pop(0))

        for grp in range(PPC // 4):
            emit_group(grp, after_pair=drain)
        while jq:
            run_job(jq.pop(0))

        # close hg accumulation: += ncount (x) b_ro
        nc.tensor.matmul(hg_ps[:], ncnt[:], b_ro_row[:],
                         start=False, stop=True, skip_group_check=True)

        # pmax = relu(max(chunk maxes) + cb4)
        pmax = wp.tile([P, PPC], bf16, tag="pmax")
        mxt = wp.tile([P, PPC], f32, tag="mxt")
        nc.vector.tensor_reduce(out=mxt[:],
                                in_=chunkmax[:].rearrange("p c q -> p q c"),
                                axis=AX.X, op=ALU.max)
        nc.scalar.activation(pmax[:], mxt[:], AF.Relu, bias=cb[3][:])
        if DEBUG_OUT:
            pmf = wp.tile([P, PPC], f32, tag="pmf")
            nc.vector.tensor_copy(pmf[:], pmax[:])
            nc.sync.dma_start(out=dbg_pmax[:], in_=pmf[:])

        # ---------------- readout + head
        hgT = wp.tile([GPC, HID], bf16, tag="hgT")
        nc.scalar.activation(hgT[:], hg_ps[:], AF.Relu)
        hgt_ps = pgn.tile([HID, GPC], bf16, space="PSUM", tag="gps")
        nc.tensor.transpose(hgt_ps[:], hgT[:], identb[:GPC, :GPC])
        hg = wp.tile([HID, GPC], bf16, tag="hg")
        nc.scalar.copy(hg[:], hgt_ps[:])
        c1ps = pgn.tile([HID, GPC], f32, space="PSUM", tag="gps")
        nc.tensor.matmul(c1ps[:], Wc1[:], hg[:], start=True, stop=True)
        cv1 = wp.tile([HID, GPC], bf16, tag="cv1")
        nc.scalar.activation(cv1[:], c1ps[:], AF.Relu, bias=bc1[:])
        c2ps = pgn.tile([HID, GPC], f32, space="PSUM", tag="gps")
        nc.tensor.matmul(c2ps[:], Wc2[:], cv1[:], start=True, stop=True)
        cv2 = wp.tile([HID, GPC], bf16, tag="cv2")
        nc.scalar.activation(cv2[:], c2ps[:], AF.Relu, bias=bc2[:])
        if DEBUG_OUT:
            cvf = wp.tile([HID, GPC], f32, tag="cvf")
            nc.vector.tensor_copy(cvf[:], cv2[:])
            nc.sync.dma_start(out=dbg_cv[:], in_=cvf[:])
        # head: z = [cv2; pmax]
        zin = [cv2, pmax]
        z2 = []
        for mc in range(2):
            zps = pgn.tile([HID, GPC], f32, space="PSUM", tag="gps")
            for kc in range(2):
                nc.tensor.matmul(zps[:], Wf1[:, kc, mc * HID:(mc + 1) * HID],
                                 zin[kc][:, :GPC], start=(kc == 0),
                                 stop=(kc == 1))
            zt = wp.tile([HID, GPC], bf16, tag="z2_%d" % mc)
            nc.scalar.activation(zt[:], zps[:], AF.Relu, bias=bf1[:, mc, :])
            z2.append(zt)
        ops = pgn.tile([1, GPC], f32, space="PSUM", tag="gps")
        for kc in range(2):
            nc.tensor.matmul(ops[:], Wf2[:, kc, :], z2[kc][:],
                             start=(kc == 0), stop=(kc == 1))
        ot = wp.tile([1, GPC], f32, tag="ot")
        nc.scalar.activation(ot[:], ops[:], AF.Sigmoid, bias=bf2[:1, :])
        nc.sync.dma_start(out=out_d[:], in_=ot[:])

    nc.compile()
    return nc


def kernel(**inputs):
    shared, percore, meta = _host_prep(inputs)
    nc = _build(shared, meta)
    in_maps = []
    for c in range(NCORES):
        m = dict(shared)
        m.update(percore[c])
        in_maps.append(m)
    res = run_bass_kernel_spmd(nc, in_maps, list(range(NCORES)))
    out = np.concatenate([res.results[c]["out"].reshape(GPC)
                          for c in range(NCORES)])
    return out.reshape(B, 1).astype(np.float32)


if __name__ == "__main__":
    sys.path.insert(0, "/root/problem")
    import jax
    import reference
    with jax.default_device(jax.devices("cpu")[0]):
        inputs = {k: np.asarray(v) for k, v in reference.setup_inputs().items()}
        exp = np.asarray(reference.reference(**inputs))
    got = kernel(**inputs)
    err = np.abs(got - exp).max()
    rel = err / max(np.abs(exp).max(), 1e-9)
    print("max abs err:", err, " rel:", rel)
